# revision 1
# baseline (speedup 1.0000x reference)
"""AxialAttention (vertical, cls token, full cls attention) Trainium2 kernel.

Strategy: data-parallel over batch (32 batches -> 8 cores x 4 batches).
Per core everything is done in one fused Bass/Tile program:
  qkv projection -> per-row axial attention (+ full cls attention) -> out proj.

Host-side layout (per batch element):
  tokens are regrouped into 7 "slots" of 113 columns: [cls, 4 rows x 28 tok].
  Everything on-chip is feature-major (C on partitions): x_T (768, 791*4).
  Row attention for slot g, head h:
    scores[112 q, 113 k] = (qT slice).T @ (kT slice) + mask-matmul (row-match
    term: +30 same-row/cls, 0 otherwise), exp(bias=-30) with accum -> Z,
    U = exp * (1/Z) (per-partition broadcast), PE-transpose -> U_T,
    Y = (v_slot).T @ U_T accumulated per chunk of 128 output features.
  cls attention: per-head logits computed transposed ([keys, head] per slot),
    exp, dup-cls rows zeroed, Z via ones-matmul, 1/Z broadcast via K=1 matmul,
    v.T @ u accumulated over slots.
  proj: out_T = W_proj.T-contract with Y_T (+ bias via ACT Identity bias).
"""

import os

os.environ.setdefault("JAX_PLATFORMS", "axon")

import sys

if "/opt/trn_rl_repo" not in sys.path:
    sys.path.insert(0, "/opt/trn_rl_repo")

import numpy as np
import ml_dtypes

import concourse.bass as bass
import concourse.bacc as bacc
import concourse.mybir as mybir
import concourse.tile as tile
from concourse.bass_utils import run_bass_kernel_spmd
from concourse.masks import make_identity

P = 128
C = 768
CH = C // P            # 6 feature chunks
NH = 12
HD = 64
HH = 28                # image H = W
ROWS = 28              # attention rows per image
RG = 4                 # rows per slot
G = 7                  # slots per batch
W = RG * HH            # 112 queries per slot
SC = W + 1             # 113 keys per slot (cls + tokens)
S = G * SC             # 791 columns per batch
NB = 4                 # batches per core
TT = NB * S            # 3164 columns per core
NCORES = 8
B_TOTAL = 32
N_TOK = 1 + ROWS * HH  # 785
MPEN = 30.0            # mask penalty

F32 = mybir.dt.float32
BF16 = mybir.dt.bfloat16
BFNP = ml_dtypes.bfloat16


def _perm_valid():
    """original-token index for each of the S slot-layout columns + validity."""
    perm = np.zeros(S, np.int64)
    valid = np.ones(S, np.bool_)
    for g in range(G):
        perm[g * SC] = 0
        if g > 0:
            valid[g * SC] = False
        for j in range(W):
            r = RG * g + j // HH   # row index (original column w)
            i = j % HH             # position in row (original row h)
            perm[g * SC + 1 + j] = 1 + i * HH + r
    return perm, valid


def _consts():
    rt = np.sqrt(MPEN)
    qext = np.zeros((RG, W), np.float32)
    for j in range(W):
        qext[j // HH, j] = rt
    kext = np.zeros((RG, SC), np.float32)
    kext[:, 0] = rt
    for j in range(W):
        kext[j // HH, 1 + j] = rt
    return qext.astype(BFNP), kext.astype(BFNP)


def build_bass():
    nc = bacc.Bacc(None, target_bir_lowering=False, debug=True)

    x_t = nc.declare_dram_parameter("x_t", [C, TT], BF16, isOutput=False)
    w_qkv = nc.declare_dram_parameter("w_qkv", [C, 3 * C], BF16, isOutput=False)
    w_proj = nc.declare_dram_parameter("w_proj", [C, C], BF16, isOutput=False)
    b_pc = nc.declare_dram_parameter("b_pc", [P, CH], F32, isOutput=False)
    qext = nc.declare_dram_parameter("qext", [RG, W], BF16, isOutput=False)
    kext = nc.declare_dram_parameter("kext", [RG, SC], BF16, isOutput=False)
    out_t = nc.declare_dram_parameter("out_t", [C, TT], F32, isOutput=True)

    NSPLIT = [(0, 512), (512, S - 512)]          # moving-dim tiling of S
    VSPLIT = [(0, 512), (512, C - 512)]          # moving-dim tiling of C (v cols)

    with tile.TileContext(nc) as tc:
        with (
            tc.tile_pool(name="const", bufs=1) as cpool,
            tc.tile_pool(name="xb", bufs=2) as xpool,
            tc.tile_pool(name="qk", bufs=2) as qkpool,
            tc.tile_pool(name="vt", bufs=2) as vpool,
            tc.tile_pool(name="yt", bufs=2) as ypool,
            tc.tile_pool(name="ob", bufs=2) as opool,
            tc.tile_pool(name="att", bufs=3) as apool,
            tc.tile_pool(name="attz", bufs=2) as zpool,
            tc.tile_pool(name="ps_mm", bufs=2, space="PSUM") as ps_mm,
            tc.tile_pool(name="ps_sc", bufs=2, space="PSUM") as ps_sc,
            tc.tile_pool(name="ps_tp", bufs=2, space="PSUM") as ps_tp,
            tc.tile_pool(name="ps_y", bufs=2, space="PSUM") as ps_y,
        ):
            # ---- constants ----
            wq_sb = cpool.tile([P, CH, 3 * C], BF16)
            nc.sync.dma_start(wq_sb[:], w_qkv.rearrange("(c p) o -> p c o", p=P))
            wp_sb = cpool.tile([P, CH, C], BF16)
            nc.sync.dma_start(wp_sb[:], w_proj.rearrange("(c p) o -> p c o", p=P))
            b_sb = cpool.tile([P, CH], F32)
            nc.sync.dma_start(b_sb[:], b_pc[:])
            qe_sb = cpool.tile([RG, W], BF16)
            nc.sync.dma_start(qe_sb[:], qext[:])
            ke_sb = cpool.tile([RG, SC], BF16)
            nc.sync.dma_start(ke_sb[:], kext[:])
            ident = cpool.tile([P, P], BF16)
            make_identity(nc, ident[:])
            onesp = cpool.tile([P, 1], BF16)
            nc.vector.memset(onesp[:], 1.0)
            onesrow = cpool.tile([1, P], F32)
            nc.vector.memset(onesrow[:], 1.0)
            negm = cpool.tile([P, 1], F32)
            nc.vector.memset(negm[:], -MPEN)

            for b in range(NB):
                cb = b * S
                # ---- load x_T for this batch ----
                xb = xpool.tile([P, CH, S], BF16, tag="xb")
                nc.sync.dma_start(
                    xb[:], x_t[:, cb : cb + S].rearrange("(c p) s -> p c s", p=P)
                )

                qT = qkpool.tile([P, CH, S], BF16, tag="qT")
                kT = qkpool.tile([P, CH, S], BF16, tag="kT")
                vtok = vpool.tile([P, G, C], BF16, tag="vtok")

                # ---- qkv projections ----
                # q and k: feature-major output [feat chunk, token]
                for o in range(2 * CH):
                    dst = qT if o < CH else kT
                    oc = o % CH
                    for (n0, nsz) in NSPLIT:
                        ps = ps_mm.tile([P, 512], F32, tag="mm")
                        for c in range(CH):
                            nc.tensor.matmul(
                                ps[:, :nsz],
                                lhsT=wq_sb[:, c, o * P : (o + 1) * P],
                                rhs=xb[:, c, n0 : n0 + nsz],
                                start=(c == 0),
                                stop=(c == CH - 1),
                            )
                        nc.scalar.copy(dst[:, oc, n0 : n0 + nsz], ps[:, :nsz])
                # v: token-major per slot [113 tokens, C]
                for g in range(G):
                    for (v0, vsz) in VSPLIT:
                        ps = ps_mm.tile([P, 512], F32, tag="mm")
                        for c in range(CH):
                            nc.tensor.matmul(
                                ps[:SC, :vsz],
                                lhsT=xb[:, c, g * SC : (g + 1) * SC],
                                rhs=wq_sb[:, c, 2 * C + v0 : 2 * C + v0 + vsz],
                                start=(c == 0),
                                stop=(c == CH - 1),
                            )
                        nc.scalar.copy(
                            vtok[:SC, g, v0 : v0 + vsz], ps[:SC, :vsz]
                        )

                yT = ypool.tile([P, CH, S], BF16, tag="yT")
                nc.vector.memset(yT[:], 0.0)

                # ---- row attention ----
                for g in range(G):
                    k0 = g * SC
                    zt = zpool.tile([W, NH], F32, tag="zt")
                    rzt = zpool.tile([W, NH], F32, tag="rzt")
                    us = []
                    for h in range(NH):
                        c, hf = h // 2, (h % 2) * HD
                        sp = ps_sc.tile([W, SC], F32, tag="sc")
                        nc.tensor.matmul(
                            sp[:],
                            lhsT=qT[hf : hf + HD, c, k0 + 1 : k0 + SC],
                            rhs=kT[hf : hf + HD, c, k0 : k0 + SC],
                            start=True,
                            stop=False,
                        )
                        nc.tensor.matmul(
                            sp[:],
                            lhsT=qe_sb[:],
                            rhs=ke_sb[:],
                            start=False,
                            stop=True,
                        )
                        u = apool.tile([W, SC], BF16, tag="u", bufs=14)
                        nc.scalar.activation(
                            u[:],
                            sp[:],
                            mybir.ActivationFunctionType.Exp,
                            bias=negm[:W, :],
                            accum_out=zt[:, h : h + 1],
                        )
                        # per-head reciprocal + normalize: subtile deps let each
                        # head's chain complete without waiting for all 12 exps
                        nc.vector.reciprocal(rzt[:, h : h + 1], zt[:, h : h + 1])
                        nc.vector.tensor_tensor(
                            u[:],
                            u[:],
                            rzt[:, h : h + 1].to_broadcast([W, SC]),
                            mybir.AluOpType.mult,
                        )
                        us.append(u)
                    uts = []
                    for h in range(NH):
                        tp = ps_tp.tile([SC, W], BF16, tag="tp")
                        nc.tensor.transpose(tp[:], us[h][:], ident[:W, :W])
                        ut = apool.tile([SC, W], BF16, tag="ut", bufs=14)
                        nc.vector.tensor_copy(ut[:], tp[:])
                        uts.append(ut)
                    for c in range(CH):
                        yp = ps_y.tile([P, W], F32, tag="y")
                        for hf in range(2):
                            h = 2 * c + hf
                            nc.tensor.matmul(
                                yp[hf * HD : (hf + 1) * HD, :],
                                lhsT=vtok[:SC, g, h * HD : (h + 1) * HD],
                                rhs=uts[h][:],
                                start=True,
                                stop=True,
                            )
                        nc.vector.tensor_copy(yT[:, c, k0 + 1 : k0 + SC], yp[:])

                # ---- cls attention (logits computed transposed: [keys, head]) ----
                ucts = []
                for g in range(G):
                    cp = ps_tp.tile([SC, NH], F32, tag="tp")
                    for h in range(NH):
                        c, hf = h // 2, (h % 2) * HD
                        nc.tensor.matmul(
                            cp[:, h : h + 1],
                            lhsT=kT[hf : hf + HD, c, g * SC : (g + 1) * SC],
                            rhs=qT[hf : hf + HD, c, 0:1],
                            start=True,
                            stop=True,
                        )
                    uct = apool.tile([SC, NH], BF16, tag="uct", bufs=9)
                    nc.scalar.activation(
                        uct[:], cp[:], mybir.ActivationFunctionType.Exp
                    )
                    if g > 0:
                        nc.vector.memset(uct[0:1, :], 0.0)
                    ucts.append(uct)
                # Z over all keys via ones-vector matmuls, accumulated per slot
                zp = ps_tp.tile([1, NH], F32, tag="tp")
                for g in range(G):
                    nc.tensor.matmul(
                        zp[:],
                        lhsT=onesp[:SC, :],
                        rhs=ucts[g][:],
                        start=(g == 0),
                        stop=(g == G - 1),
                    )
                zcs = zpool.tile([1, NH], F32, tag="zcs")
                nc.vector.tensor_copy(zcs[:], zp[:])
                rzc = zpool.tile([1, NH], F32, tag="rzc")
                nc.vector.reciprocal(rzc[:], zcs[:])
                # broadcast 1/Z to all partitions via a K=1 matmul with ones
                rzb = ps_tp.tile([P, NH], F32, tag="tp")
                nc.tensor.matmul(
                    rzb[:], lhsT=onesrow[:], rhs=rzc[:], start=True, stop=True
                )
                for g in range(G):
                    nc.vector.tensor_tensor(
                        ucts[g][:], ucts[g][:], rzb[:SC, :], mybir.AluOpType.mult
                    )
                for c in range(CH):
                    yp = ps_y.tile([P, W], F32, tag="y")
                    for hf in range(2):
                        h = 2 * c + hf
                        for g in range(G):
                            nc.tensor.matmul(
                                yp[hf * HD : (hf + 1) * HD, 0:1],
                                lhsT=vtok[:SC, g, h * HD : (h + 1) * HD],
                                rhs=ucts[g][:, h : h + 1],
                                start=(g == 0),
                                stop=(g == G - 1),
                            )
                    nc.vector.tensor_copy(yT[:, c, 0:1], yp[:, 0:1])

                # ---- output projection ----
                ob = opool.tile([P, CH, S], F32, tag="ob")
                for o in range(CH):
                    for (n0, nsz) in NSPLIT:
                        ps = ps_mm.tile([P, 512], F32, tag="mm")
                        for c in range(CH):
                            nc.tensor.matmul(
                                ps[:, :nsz],
                                lhsT=wp_sb[:, c, o * P : (o + 1) * P],
                                rhs=yT[:, c, n0 : n0 + nsz],
                                start=(c == 0),
                                stop=(c == CH - 1),
                            )
                        nc.scalar.activation(
                            ob[:, o, n0 : n0 + nsz],
                            ps[:, :nsz],
                            mybir.ActivationFunctionType.Identity,
                            bias=b_sb[:, o : o + 1],
                        )
                nc.sync.dma_start(
                    out_t[:, cb : cb + S].rearrange("(c p) s -> p c s", p=P), ob[:]
                )

    nc.compile()
    return nc


_NC_CACHE = None
_LAST_IN_MAPS = None


def kernel(x, w_qkv, w_proj, b_proj):
    global _NC_CACHE, _LAST_IN_MAPS
    x = np.asarray(x)
    w_qkv = np.asarray(w_qkv)
    w_proj = np.asarray(w_proj)
    b_proj = np.asarray(b_proj)

    perm, valid = _perm_valid()
    qext, kext = _consts()

    wq = np.array(w_qkv, np.float32, copy=True)
    wq[:, :C] *= 1.0 / np.sqrt(HD)
    wq = wq.astype(BFNP)
    wp = w_proj.astype(BFNP)
    b_pc = np.ascontiguousarray(b_proj.astype(np.float32).reshape(CH, P).T)

    in_maps = []
    for core in range(NCORES):
        xs = x[core * NB : (core + 1) * NB]          # (NB, 785, C)
        xp = xs[:, perm, :]                          # (NB, S, C)
        x_T = np.ascontiguousarray(
            xp.transpose(2, 0, 1).reshape(C, TT)
        ).astype(BFNP)
        in_maps.append(
            {
                "x_t": x_T,
                "w_qkv": wq,
                "w_proj": wp,
                "b_pc": b_pc,
                "qext": qext,
                "kext": kext,
            }
        )

    if _NC_CACHE is None:
        _NC_CACHE = build_bass()
    nc = _NC_CACHE

    _LAST_IN_MAPS = in_maps

    res = run_bass_kernel_spmd(nc, in_maps, core_ids=list(range(NCORES)))

    out = np.zeros((B_TOTAL, N_TOK, C), np.float32)
    vperm = perm[valid]
    for core in range(NCORES):
        o_t = res.results[core]["out_t"]             # (C, TT) f32
        op = o_t.reshape(C, NB, S).transpose(1, 2, 0)  # (NB, S, C)
        out[core * NB : (core + 1) * NB][:, vperm, :] = op[:, valid, :]
    return out


if __name__ == "__main__":
    rng = np.random.default_rng(0)
    x = rng.standard_normal((B_TOTAL, N_TOK, C)).astype(np.float32)
    w_qkv = (rng.standard_normal((C, 3 * C)) * 0.02).astype(np.float32)
    w_proj = (rng.standard_normal((C, C)) * 0.02).astype(np.float32)
    b_proj = np.zeros((C,), np.float32)
    y = kernel(x=x, w_qkv=w_qkv, w_proj=w_proj, b_proj=b_proj)
    print(y.shape, y.dtype)



# revision 22
# speedup vs baseline: 1.1656x; 1.1656x over previous
"""AxialAttention (vertical, cls token, full cls attention) Trainium2 kernel.

Data-parallel over batch (32 batches -> 8 cores x 4 batches). Per core one
fused Bass/Tile program: qkv projection -> axial attention -> out projection.

Layout per batch: tokens regrouped into 7 slots of 113 columns
[cls, 4 rows x 28 tokens]; on-chip feature-major x_T (768, 791*4).

Attention is blocked per (slot g, head-block e) where each block holds 3
heads of equal parity (even heads live at partitions 0-63 of their feature
chunk, odd at 64-127), so every matmul in a block's PSUM accumulation group
uses the same PE quadrant (mixing tile positions within one bank group
faults real hardware). Per block:
  sp3[113 q, 3*113 k]: mask matmul (rank-5: +30 row-match terms, a cls-kill
  term for dup-cls keys) opens the bank group; 3 per-head score matmuls
  accumulate, last one closes. ONE batched exp (bias 0 for the cls query
  row, -30 for token rows) -> u3 bf16. DVE segmented tensor_reduce -> Z per
  (query, head). After a slot's 4 blocks: rz = 1/Z (cls row forced to 1),
  u3 token rows normalized in place, PE transposes per head into one PSUM
  tile, ONE DVE copy -> ut3. AV matmuls then need no transpose and yT
  writes are plain DVE copies. The cls query rides as column 0 of each
  block: its unnormalized weights are saved (ucls) and combined at batch
  end with the cross-slot Z sum into the cls output column.

Engines: PE matmuls; ACT exps + psum->sbuf copies (+bias); DVE reduces,
ut copies, yT writes; Pool(gpsimd) u3 normalize + small SBUF ops.
Output DMA'd as bf16, converted on host.
"""

import os

os.environ.setdefault("JAX_PLATFORMS", "axon")

import sys

if "/opt/trn_rl_repo" not in sys.path:
    sys.path.insert(0, "/opt/trn_rl_repo")

import numpy as np
import ml_dtypes

import concourse.bass as bass
import concourse.bacc as bacc
import concourse.mybir as mybir
import concourse.tile as tile
from concourse.bass_utils import run_bass_kernel_spmd
from concourse.masks import make_identity

P = 128
C = 768
CH = C // P            # 6 feature chunks
NH = 12
HD = 64
HH = 28                # image H = W
RG = 4                 # rows per slot
G = 7                  # slots per batch
W = RG * HH            # 112 token queries per slot
SC = W + 1             # 113 columns per slot (cls + tokens)
S = G * SC             # 791 columns per batch
NB = 4                 # batches per core
TT = NB * S            # 3164 columns per core
NCORES = 8
B_TOTAL = 32
N_TOK = 1 + HH * HH    # 785
MPEN = 30.0            # mask magnitude
KILL = -60.0           # dup-cls key kill (cls query, slots g>0)
NBLK = 4               # head blocks per slot (2 even-parity + 2 odd-parity)
BH = NH // NBLK        # heads per block = 3

F32 = mybir.dt.float32
BF16 = mybir.dt.bfloat16
BFNP = ml_dtypes.bfloat16

NORM_ENG = os.environ.get("BASSK_NORM_ENGINE", "pool")
SMALL_ENG = os.environ.get("BASSK_SMALL_ENGINE", "pool")


def head_of(e, hh):
    """block (e, hh) -> head index; blocks 0,1 = even heads, 2,3 = odd."""
    if e < 2:
        return 2 * (e * BH + hh)
    return 2 * ((e - 2) * BH + hh) + 1


def blk_of(h):
    """head -> (block e, lane hh)."""
    i = h // 2
    if h % 2 == 0:
        return i // BH, i % BH
    return 2 + i // BH, i % BH


def _perm_valid():
    """original-token index for each of the S slot-layout columns + validity."""
    perm = np.zeros(S, np.int64)
    valid = np.ones(S, np.bool_)
    for g in range(G):
        perm[g * SC] = 0
        if g > 0:
            valid[g * SC] = False
        for j in range(W):
            r = RG * g + j // HH   # row index (original column w)
            i = j % HH             # position in row (original row h)
            perm[g * SC + 1 + j] = 1 + i * HH + r
    return perm, valid


def _consts():
    rt = np.sqrt(MPEN)
    qe = np.zeros((5, SC), np.float32)
    ke1 = np.zeros((5, SC), np.float32)
    for j in range(W):
        qe[j // HH, 1 + j] = rt
        ke1[j // HH, 1 + j] = rt
    ke1[:RG, 0] = rt
    ke1[4, 0] = 1.0
    qen = np.zeros((P, SC), np.float32)
    qen[0:5] = qe
    qen[64:69] = qe
    qed = qen.copy()
    qed[4, 0] = KILL
    qed[68, 0] = KILL
    ke5 = np.zeros((P, BH * SC), np.float32)
    ke5[0:5] = np.tile(ke1, (1, BH))
    ke5[64:69] = ke5[0:5]
    biasq = np.full((SC, 1), -MPEN, np.float32)
    biasq[0, 0] = 0.0
    selc = np.zeros((NH, CH * P), np.float32)
    for h in range(NH):
        c, hf = h // 2, h % 2
        selc[h, c * P + hf * HD : c * P + (hf + 1) * HD] = 1.0
    return (
        qen.astype(BFNP),
        qed.astype(BFNP),
        ke5.astype(BFNP),
        biasq,
        selc.astype(BFNP),
    )


def build_bass():
    nc = bacc.Bacc(None, target_bir_lowering=False, debug=True)

    x_t = nc.declare_dram_parameter("x_t", [C, TT], BF16, isOutput=False)
    w_qkv = nc.declare_dram_parameter("w_qkv", [C, 3 * C], BF16, isOutput=False)
    w_proj = nc.declare_dram_parameter("w_proj", [C, C], BF16, isOutput=False)
    b_pc = nc.declare_dram_parameter("b_pc", [P, CH], F32, isOutput=False)
    qen_d = nc.declare_dram_parameter("qen", [P, SC], BF16, isOutput=False)
    qed_d = nc.declare_dram_parameter("qed", [P, SC], BF16, isOutput=False)
    ke5_d = nc.declare_dram_parameter("ke5", [P, BH * SC], BF16, isOutput=False)
    biasq_d = nc.declare_dram_parameter("biasq", [SC, 1], F32, isOutput=False)
    selc_d = nc.declare_dram_parameter("selc", [NH, CH * P], BF16, isOutput=False)
    out_t = nc.declare_dram_parameter("out_t", [C, TT], BF16, isOutput=True)

    NSPLIT = [(0, 512), (512, S - 512)]          # moving-dim tiling of S
    VSPLIT = [(0, 512), (512, C - 512)]          # moving-dim tiling of C (v cols)

    with tile.TileContext(nc) as tc:
        with (
            tc.tile_pool(name="const", bufs=1) as cpool,
            tc.tile_pool(name="xb", bufs=2) as xpool,
            tc.tile_pool(name="qk", bufs=2) as qkpool,
            tc.tile_pool(name="vt", bufs=2) as vpool,
            tc.tile_pool(name="yt", bufs=2) as ypool,
            tc.tile_pool(name="ob", bufs=2) as opool,
            tc.tile_pool(name="us", bufs=5) as upool,
            tc.tile_pool(name="zs", bufs=2) as zpool,
            tc.tile_pool(name="ps_mm", bufs=2, space="PSUM") as ps_mm,
            tc.tile_pool(name="ps_sc", bufs=2, space="PSUM") as ps_sc,
            tc.tile_pool(name="ps_tp", bufs=2, space="PSUM") as ps_tp,
            tc.tile_pool(name="ps_yz", bufs=2, space="PSUM") as ps_yz,
        ):
            # ---- constants ----
            wq_sb = cpool.tile([P, CH, 3 * C], BF16)
            wp_sb = cpool.tile([P, CH, C], BF16)
            wq_r = w_qkv.rearrange("(c p) o -> p c o", p=P)
            wp_r = w_proj.rearrange("(c p) o -> p c o", p=P)
            for c in range(CH):
                nc.sync.dma_start(wq_sb[:, c, :], wq_r[:, c, :])
            for c in range(CH):
                nc.sync.dma_start(wp_sb[:, c, :], wp_r[:, c, :])
            b_sb = cpool.tile([P, CH], F32)
            nc.sync.dma_start(b_sb[:], b_pc[:])
            qen = cpool.tile([P, SC], BF16)
            nc.sync.dma_start(qen[:], qen_d[:])
            qed = cpool.tile([P, SC], BF16)
            nc.sync.dma_start(qed[:], qed_d[:])
            ke5 = cpool.tile([P, BH * SC], BF16)
            nc.sync.dma_start(ke5[:], ke5_d[:])
            biasq = cpool.tile([SC, 1], F32)
            nc.sync.dma_start(biasq[:], biasq_d[:])
            selc = cpool.tile([NH, CH * P], BF16)
            nc.sync.dma_start(selc[:], selc_d[:])
            identb = cpool.tile([P, P], BF16)
            make_identity(nc, identb[:])
            identf = cpool.tile([P, P], F32)
            make_identity(nc, identf[:])

            norm_eng = {"pool": nc.gpsimd, "vector": nc.vector}[NORM_ENG]
            small_eng = {"pool": nc.gpsimd, "vector": nc.vector}[SMALL_ENG]

            def emit_qkv(xb, qT, kT, vtok):
                # q and k: feature-major output [feat chunk, token]
                for o in range(2 * CH):
                    dst = qT if o < CH else kT
                    oc = o % CH
                    for (n0, nsz) in NSPLIT:
                        ps = ps_mm.tile([P, 512], F32, tag="mm")
                        for c in range(CH):
                            nc.tensor.matmul(
                                ps[:, :nsz],
                                lhsT=wq_sb[:, c, o * P : (o + 1) * P],
                                rhs=xb[:, c, n0 : n0 + nsz],
                                start=(c == 0),
                                stop=(c == CH - 1),
                            )
                        nc.scalar.copy(dst[:, oc, n0 : n0 + nsz], ps[:, :nsz])
                # v: token-major per slot [113 tokens, C]
                for g in range(G):
                    for (v0, vsz) in VSPLIT:
                        ps = ps_mm.tile([P, 512], F32, tag="mm")
                        for c in range(CH):
                            nc.tensor.matmul(
                                ps[:SC, :vsz],
                                lhsT=xb[:, c, g * SC : (g + 1) * SC],
                                rhs=wq_sb[:, c, 2 * C + v0 : 2 * C + v0 + vsz],
                                start=(c == 0),
                                stop=(c == CH - 1),
                            )
                        nc.scalar.copy(vtok[:SC, g, v0 : v0 + vsz], ps[:SC, :vsz])

            def emit_out_proj(yT, ob, cb):
                for o in range(CH):
                    for (n0, nsz) in NSPLIT:
                        ps = ps_mm.tile([P, 512], F32, tag="mm")
                        for c in range(CH):
                            nc.tensor.matmul(
                                ps[:, :nsz],
                                lhsT=wp_sb[:, c, o * P : (o + 1) * P],
                                rhs=yT[:, c, n0 : n0 + nsz],
                                start=(c == 0),
                                stop=(c == CH - 1),
                            )
                        nc.scalar.activation(
                            ob[:, o, n0 : n0 + nsz],
                            ps[:, :nsz],
                            mybir.ActivationFunctionType.Identity,
                            bias=b_sb[:, o : o + 1],
                        )
                nc.sync.dma_start(
                    out_t[:, cb : cb + S].rearrange("(c p) s -> p c s", p=P), ob[:]
                )

            prev = None
            for b in range(NB):
                cb = b * S
                xb = xpool.tile([P, CH, S], BF16, tag="xb")
                nc.sync.dma_start(
                    xb[:], x_t[:, cb : cb + S].rearrange("(c p) s -> p c s", p=P)
                )

                qT = qkpool.tile([P, CH, S], BF16, tag="qT")
                kT = qkpool.tile([P, CH, S], BF16, tag="kT")
                vtok = vpool.tile([P, G, C], BF16, tag="vtok")
                emit_qkv(xb, qT, kT, vtok)

                if prev is not None:
                    emit_out_proj(*prev)
                    prev = None

                # ---- attention ----
                yT = ypool.tile([P, CH, S], BF16, tag="yT")
                zt = zpool.tile([SC, G, NH], F32, tag="zt")
                ucls = zpool.tile([SC, G, NH], BF16, tag="ucls")
                sps = {}   # block t -> sp3 psum tile
                us = {}    # block t -> u3 sbuf tile (exp'd scores)
                uts = {}   # block t -> ut3 sbuf tile

                def stageA(t, qT=qT, kT=kT, sps=sps):
                    g, e = t // NBLK, t % NBLK
                    k0 = g * SC
                    pq = 0 if e < 2 else HD    # PE quadrant of this block
                    sp3 = ps_sc.tile([SC, BH, SC], F32, tag="sc")
                    qe = qen if g == 0 else qed
                    # one accumulation group per PSUM bank: mask opens it over
                    # the whole tile, per-head scores accumulate, last closes.
                    # All matmuls of the block use the same PE quadrant.
                    nc.tensor.matmul(
                        sp3[:, :, :],
                        lhsT=qe[pq : pq + HD, :],
                        rhs=ke5[pq : pq + HD, :],
                        start=True,
                        stop=False,
                    )
                    for hh in range(BH):
                        h = head_of(e, hh)
                        c = h // 2
                        nc.tensor.matmul(
                            sp3[:, hh, :],
                            lhsT=qT[pq : pq + HD, c, k0 : k0 + SC],
                            rhs=kT[pq : pq + HD, c, k0 : k0 + SC],
                            start=False,
                            stop=(hh == BH - 1),
                        )
                    sps[t] = sp3

                def stageB1(t, zt=zt, sps=sps, us=us):
                    g, e = t // NBLK, t % NBLK
                    sp3 = sps.pop(t)
                    u3 = upool.tile([SC, BH, SC], BF16, tag="u3")
                    nc.scalar.activation(
                        u3[:, :, :],
                        sp3[:, :, :],
                        mybir.ActivationFunctionType.Exp,
                        bias=biasq[:],
                    )
                    nc.vector.tensor_reduce(
                        zt[:, g, e * BH : (e + 1) * BH],
                        u3[:, :, :],
                        mybir.AxisListType.X,
                        mybir.AluOpType.add,
                    )
                    us[t] = u3

                def stageB2(g, rzb, ucls=ucls, us=us, uts=uts):
                    # normalize token-query rows in place (rzb row 0 is 1.0 so
                    # the cls row stays raw for the cross-slot cls softmax),
                    # then transpose per head
                    for e in range(NBLK):
                        t = g * NBLK + e
                        u3 = us.pop(t)
                        norm_eng.tensor_tensor(
                            u3[:, :, :],
                            u3[:, :, :],
                            rzb[:, e * BH : (e + 1) * BH].to_broadcast(
                                [SC, BH, SC]
                            ),
                            mybir.AluOpType.mult,
                        )
                        tp3 = ps_tp.tile([SC, BH, P], BF16, tag="tp")
                        for hh in range(BH):
                            nc.tensor.transpose(
                                tp3[:, hh, :SC], u3[:, hh, :], identb[:SC, :SC]
                            )
                        ut3 = upool.tile([SC, BH, SC], BF16, tag="ut3", bufs=5)
                        nc.vector.tensor_copy(ut3[:, :, :], tp3[:, :, :SC])
                        small_eng.tensor_copy(
                            ucls[:, g, e * BH : (e + 1) * BH], ut3[:, :, 0:1]
                        )
                        uts[t] = ut3

                def stageC(g, yT=yT, vtok=vtok, uts=uts):
                    k0 = g * SC
                    ut_blk = [uts.pop(g * NBLK + e) for e in range(NBLK)]
                    for c in range(CH):
                        yz = ps_yz.tile([P, SC], F32, tag="yz")
                        for hf in range(2):
                            h = 2 * c + hf
                            e, hh = blk_of(h)
                            nc.tensor.matmul(
                                yz[hf * HD : (hf + 1) * HD, 0:SC],
                                lhsT=vtok[:SC, g, h * HD : (h + 1) * HD],
                                rhs=ut_blk[e][:, hh, :],
                                start=True,
                                stop=True,
                            )
                        nc.vector.tensor_copy(
                            yT[:, c, k0 + 1 : k0 + SC], yz[:, 1:SC]
                        )

                NT = G * NBLK
                for t in range(NT + 1):
                    if t < NT:
                        stageA(t)
                    if t >= 1:
                        stageB1(t - 1)
                        if (t - 1) % NBLK == NBLK - 1:
                            g = (t - 1) // NBLK
                            rzf = zpool.tile([SC, NH], F32, tag="rzf")
                            nc.vector.reciprocal(rzf[:], zt[:, g, :])
                            rzb = zpool.tile([SC, NH], BF16, tag="rzb")
                            small_eng.tensor_copy(rzb[:], rzf[:])
                            small_eng.memset(rzb[0:1, :], 1.0)
                            stageB2(g, rzb)
                            stageC(g)

                # ---- cls finalize ----
                zcls = zpool.tile([1, NH], F32, tag="zcls")
                small_eng.tensor_copy(zcls[:], zt[0:1, 0, :])
                for g in range(1, G):
                    small_eng.tensor_tensor(
                        zcls[:], zcls[:], zt[0:1, g, :], mybir.AluOpType.add
                    )
                rzc = zpool.tile([1, NH], F32, tag="rzc")
                nc.vector.reciprocal(rzc[:], zcls[:])
                rzcp = ps_tp.tile([NH, 1], F32, tag="tp")
                nc.tensor.transpose(rzcp[:], rzc[:], identf[:1, :1])
                rzcT = zpool.tile([NH, 1], BF16, tag="rzcT")
                nc.vector.tensor_copy(rzcT[:], rzcp[:])
                for c in range(CH):
                    ycls = ps_yz.tile([P, SC], F32, tag="yz")
                    for g in range(G):
                        for hf in range(2):
                            h = 2 * c + hf
                            e, hh = blk_of(h)
                            nc.tensor.matmul(
                                ycls[hf * HD : (hf + 1) * HD, 0:1],
                                lhsT=vtok[:SC, g, h * HD : (h + 1) * HD],
                                rhs=ucls[:, g, e * BH + hh : e * BH + hh + 1],
                                start=(g == 0),
                                stop=(g == G - 1),
                            )
                    rzbc = ps_tp.tile([P, 1], F32, tag="tp")
                    nc.tensor.matmul(
                        rzbc[:],
                        lhsT=selc[:, c * P : (c + 1) * P],
                        rhs=rzcT[:],
                        start=True,
                        stop=True,
                    )
                    rzbc_sb = zpool.tile([P, 1], BF16, tag="rzbc")
                    nc.vector.tensor_copy(rzbc_sb[:], rzbc[:])
                    nc.vector.tensor_tensor(
                        yT[:, c, 0:1],
                        ycls[:, 0:1],
                        rzbc_sb[:],
                        mybir.AluOpType.mult,
                    )

                ob = opool.tile([P, CH, S], BF16, tag="ob")
                prev = (yT, ob, cb)

            emit_out_proj(*prev)

    nc.compile()
    return nc


_NC_CACHE = None
_LAST_IN_MAPS = None


def kernel(x, w_qkv, w_proj, b_proj):
    global _NC_CACHE, _LAST_IN_MAPS
    x = np.asarray(x)
    w_qkv = np.asarray(w_qkv)
    w_proj = np.asarray(w_proj)
    b_proj = np.asarray(b_proj)

    perm, valid = _perm_valid()
    qen, qed, ke5, biasq, selc = _consts()

    wq = np.array(w_qkv, np.float32, copy=True)
    wq[:, :C] *= 1.0 / np.sqrt(HD)
    wq = wq.astype(BFNP)
    wp = w_proj.astype(BFNP)
    b_pc = np.ascontiguousarray(b_proj.astype(np.float32).reshape(CH, P).T)

    in_maps = []
    for core in range(NCORES):
        xs = x[core * NB : (core + 1) * NB]          # (NB, 785, C)
        xp = xs[:, perm, :]                          # (NB, S, C)
        x_T = np.ascontiguousarray(
            xp.transpose(2, 0, 1).reshape(C, TT)
        ).astype(BFNP)
        in_maps.append(
            {
                "x_t": x_T,
                "w_qkv": wq,
                "w_proj": wp,
                "b_pc": b_pc,
                "qen": qen,
                "qed": qed,
                "ke5": ke5,
                "biasq": biasq,
                "selc": selc,
            }
        )

    if _NC_CACHE is None:
        _NC_CACHE = build_bass()
    nc = _NC_CACHE

    _LAST_IN_MAPS = in_maps

    res = run_bass_kernel_spmd(nc, in_maps, core_ids=list(range(NCORES)))

    out = np.zeros((B_TOTAL, N_TOK, C), np.float32)
    vperm = perm[valid]
    for core in range(NCORES):
        o_t = np.asarray(res.results[core]["out_t"]).astype(np.float32)
        op = o_t.reshape(C, NB, S).transpose(1, 2, 0)  # (NB, S, C)
        out[core * NB : (core + 1) * NB][:, vperm, :] = op[:, valid, :]
    return out


if __name__ == "__main__":
    rng = np.random.default_rng(0)
    x = rng.standard_normal((B_TOTAL, N_TOK, C)).astype(np.float32)
    w_qkv = (rng.standard_normal((C, 3 * C)) * 0.02).astype(np.float32)
    w_proj = (rng.standard_normal((C, C)) * 0.02).astype(np.float32)
    b_proj = np.zeros((C,), np.float32)
    y = kernel(x=x, w_qkv=w_qkv, w_proj=w_proj, b_proj=b_proj)
    print(np.abs(y).mean(), y.shape, y.dtype)


# revision 23
# speedup vs baseline: 1.3658x; 1.1718x over previous
"""AxialAttention (vertical, cls token, full cls attention) Trainium2 kernel.

Data-parallel over batch (32 batches -> 8 cores x 4 batches). Per core one
fused Bass/Tile program: qkv projection -> axial attention -> out projection.

Layout per batch: tokens regrouped into 7 slots of 113 columns
[cls, 4 rows x 28 tokens]; on-chip feature-major x_T (768, 791*4).

Attention is blocked per (slot g, head-block e) where each block holds 3
heads of equal parity (even heads live at partitions 0-63 of their feature
chunk, odd at 64-127), so every matmul in a block's PSUM accumulation group
uses the same PE quadrant (mixing tile positions within one bank group
faults real hardware). Per block:
  sp3[113 q, 3*113 k]: mask matmul (rank-5: +30 row-match terms, a cls-kill
  term for dup-cls keys) opens the bank group; 3 per-head score matmuls
  accumulate, last one closes. ONE batched exp (bias 0 for the cls query
  row, -30 for token rows) -> u3 bf16. DVE segmented tensor_reduce -> Z per
  (query, head). After a slot's 4 blocks: rz = 1/Z (cls row forced to 1),
  u3 token rows normalized in place, PE transposes per head into one PSUM
  tile, ONE DVE copy -> ut3. AV matmuls then need no transpose and yT
  writes are plain DVE copies. The cls query rides as column 0 of each
  block: its unnormalized weights are saved (ucls) and combined at batch
  end with the cross-slot Z sum into the cls output column.

Engines: PE matmuls; ACT exps + psum->sbuf copies (+bias); DVE reduces,
ut copies, yT writes; Pool(gpsimd) u3 normalize + small SBUF ops.
Output DMA'd as bf16, converted on host.
"""

import os

os.environ.setdefault("JAX_PLATFORMS", "axon")

import sys

if "/opt/trn_rl_repo" not in sys.path:
    sys.path.insert(0, "/opt/trn_rl_repo")

import numpy as np
import ml_dtypes

import concourse.bass as bass
import concourse.bacc as bacc
import concourse.mybir as mybir
import concourse.tile as tile
from concourse.bass_utils import run_bass_kernel_spmd
from concourse.masks import make_identity

P = 128
C = 768
CH = C // P            # 6 feature chunks
NH = 12
HD = 64
HH = 28                # image H = W
RG = 4                 # rows per slot
G = 7                  # slots per batch
W = RG * HH            # 112 token queries per slot
SC = W + 1             # 113 columns per slot (cls + tokens)
S = G * SC             # 791 columns per batch
NB = 4                 # batches per core
TT = NB * S            # 3164 columns per core
NCORES = 8
B_TOTAL = 32
N_TOK = 1 + HH * HH    # 785
MPEN = 30.0            # mask magnitude
KILL = -60.0           # dup-cls key kill (cls query, slots g>0)
NBLK = 4               # head blocks per slot (2 even-parity + 2 odd-parity)
BH = NH // NBLK        # heads per block = 3

F32 = mybir.dt.float32
BF16 = mybir.dt.bfloat16
BFNP = ml_dtypes.bfloat16

NORM_ENG = os.environ.get("BASSK_NORM_ENGINE", "pool")
SMALL_ENG = os.environ.get("BASSK_SMALL_ENGINE", "pool")


def head_of(e, hh):
    """block (e, hh) -> head index; blocks 0,1 = even heads, 2,3 = odd."""
    if e < 2:
        return 2 * (e * BH + hh)
    return 2 * ((e - 2) * BH + hh) + 1


def blk_of(h):
    """head -> (block e, lane hh)."""
    i = h // 2
    if h % 2 == 0:
        return i // BH, i % BH
    return 2 + i // BH, i % BH


def _perm_valid():
    """original-token index for each of the S slot-layout columns + validity."""
    perm = np.zeros(S, np.int64)
    valid = np.ones(S, np.bool_)
    for g in range(G):
        perm[g * SC] = 0
        if g > 0:
            valid[g * SC] = False
        for j in range(W):
            r = RG * g + j // HH   # row index (original column w)
            i = j % HH             # position in row (original row h)
            perm[g * SC + 1 + j] = 1 + i * HH + r
    return perm, valid


def _consts():
    rt = np.sqrt(MPEN)
    qe = np.zeros((5, SC), np.float32)
    ke1 = np.zeros((5, SC), np.float32)
    for j in range(W):
        qe[j // HH, 1 + j] = rt
        ke1[j // HH, 1 + j] = rt
    ke1[:RG, 0] = rt
    ke1[4, 0] = 1.0
    qen = np.zeros((P, SC), np.float32)
    qen[0:5] = qe
    qen[64:69] = qe
    qed = qen.copy()
    qed[4, 0] = KILL
    qed[68, 0] = KILL
    ke5 = np.zeros((P, BH * SC), np.float32)
    ke5[0:5] = np.tile(ke1, (1, BH))
    ke5[64:69] = ke5[0:5]
    biasq = np.full((SC, 1), -MPEN, np.float32)
    biasq[0, 0] = 0.0
    selc = np.zeros((NH, CH * P), np.float32)
    for h in range(NH):
        c, hf = h // 2, h % 2
        selc[h, c * P + hf * HD : c * P + (hf + 1) * HD] = 1.0
    return (
        qen.astype(BFNP),
        qed.astype(BFNP),
        ke5.astype(BFNP),
        biasq,
        selc.astype(BFNP),
    )


def build_bass():
    nc = bacc.Bacc(None, target_bir_lowering=False, debug=True)

    x_t = nc.declare_dram_parameter("x_t", [C, TT], BF16, isOutput=False)
    w_qkv = nc.declare_dram_parameter("w_qkv", [C, 3 * C], BF16, isOutput=False)
    w_proj = nc.declare_dram_parameter("w_proj", [C, C], BF16, isOutput=False)
    b_pc = nc.declare_dram_parameter("b_pc", [P, CH], F32, isOutput=False)
    qen_d = nc.declare_dram_parameter("qen", [P, SC], BF16, isOutput=False)
    qed_d = nc.declare_dram_parameter("qed", [P, SC], BF16, isOutput=False)
    ke5_d = nc.declare_dram_parameter("ke5", [P, BH * SC], BF16, isOutput=False)
    biasq_d = nc.declare_dram_parameter("biasq", [SC, 1], F32, isOutput=False)
    selc_d = nc.declare_dram_parameter("selc", [NH, CH * P], BF16, isOutput=False)
    out_t = nc.declare_dram_parameter("out_t", [C, TT], BF16, isOutput=True)

    NSPLIT = [(0, 512), (512, S - 512)]          # moving-dim tiling of S
    VSPLIT = [(0, 512), (512, C - 512)]          # moving-dim tiling of C (v cols)

    with tile.TileContext(nc) as tc:
        with (
            tc.tile_pool(name="const", bufs=1) as cpool,
            tc.tile_pool(name="xb", bufs=2) as xpool,
            tc.tile_pool(name="qk", bufs=2) as qkpool,
            tc.tile_pool(name="vt", bufs=2) as vpool,
            tc.tile_pool(name="yt", bufs=2) as ypool,
            tc.tile_pool(name="ob", bufs=2) as opool,
            tc.tile_pool(name="us", bufs=9) as upool,
            tc.tile_pool(name="zs", bufs=2) as zpool,
            tc.tile_pool(name="ps_mm", bufs=2, space="PSUM") as ps_mm,
            tc.tile_pool(name="ps_sc", bufs=2, space="PSUM") as ps_sc,
            tc.tile_pool(name="ps_tp", bufs=2, space="PSUM") as ps_tp,
            tc.tile_pool(name="ps_yz", bufs=2, space="PSUM") as ps_yz,
        ):
            # ---- constants ----
            wq_sb = cpool.tile([P, CH, 3 * C], BF16)
            wp_sb = cpool.tile([P, CH, C], BF16)
            wq_r = w_qkv.rearrange("(c p) o -> p c o", p=P)
            wp_r = w_proj.rearrange("(c p) o -> p c o", p=P)
            nc.sync.dma_start(wq_sb[:, 0, :], wq_r[:, 0, :])
            xb0 = xpool.tile([P, CH, S], BF16, tag="xb")
            nc.sync.dma_start(
                xb0[:], x_t[:, 0:S].rearrange("(c p) s -> p c s", p=P)
            )
            for c in range(1, CH):
                nc.sync.dma_start(wq_sb[:, c, :], wq_r[:, c, :])
            b_sb = cpool.tile([P, CH], F32)
            nc.sync.dma_start(b_sb[:], b_pc[:])
            qen = cpool.tile([P, SC], BF16)
            nc.sync.dma_start(qen[:], qen_d[:])
            qed = cpool.tile([P, SC], BF16)
            nc.sync.dma_start(qed[:], qed_d[:])
            ke5 = cpool.tile([P, BH * SC], BF16)
            nc.sync.dma_start(ke5[:], ke5_d[:])
            biasq = cpool.tile([SC, 1], F32)
            nc.sync.dma_start(biasq[:], biasq_d[:])
            selc = cpool.tile([NH, CH * P], BF16)
            nc.sync.dma_start(selc[:], selc_d[:])
            identb = cpool.tile([P, P], BF16)
            make_identity(nc, identb[:])
            identf = cpool.tile([P, P], F32)
            make_identity(nc, identf[:])

            norm_eng = {"pool": nc.gpsimd, "vector": nc.vector}[NORM_ENG]
            small_eng = {"pool": nc.gpsimd, "vector": nc.vector}[SMALL_ENG]

            def emit_qkv(xb, qT, kT, vtok):
                # q and k: feature-major output [feat chunk, token]
                for o in range(2 * CH):
                    dst = qT if o < CH else kT
                    oc = o % CH
                    for (n0, nsz) in NSPLIT:
                        ps = ps_mm.tile([P, 512], F32, tag="mm")
                        for c in range(CH):
                            nc.tensor.matmul(
                                ps[:, :nsz],
                                lhsT=wq_sb[:, c, o * P : (o + 1) * P],
                                rhs=xb[:, c, n0 : n0 + nsz],
                                start=(c == 0),
                                stop=(c == CH - 1),
                            )
                        nc.scalar.copy(dst[:, oc, n0 : n0 + nsz], ps[:, :nsz])
                # v: token-major per slot [113 tokens, C]
                for g in range(G):
                    for (v0, vsz) in VSPLIT:
                        ps = ps_mm.tile([P, 512], F32, tag="mm")
                        for c in range(CH):
                            nc.tensor.matmul(
                                ps[:SC, :vsz],
                                lhsT=xb[:, c, g * SC : (g + 1) * SC],
                                rhs=wq_sb[:, c, 2 * C + v0 : 2 * C + v0 + vsz],
                                start=(c == 0),
                                stop=(c == CH - 1),
                            )
                        nc.scalar.copy(vtok[:SC, g, v0 : v0 + vsz], ps[:SC, :vsz])

            def emit_out_proj(yT, ob, cb):
                for o in range(CH):
                    for (n0, nsz) in NSPLIT:
                        ps = ps_mm.tile([P, 512], F32, tag="mm")
                        for c in range(CH):
                            nc.tensor.matmul(
                                ps[:, :nsz],
                                lhsT=wp_sb[:, c, o * P : (o + 1) * P],
                                rhs=yT[:, c, n0 : n0 + nsz],
                                start=(c == 0),
                                stop=(c == CH - 1),
                            )
                        nc.scalar.activation(
                            ob[:, o, n0 : n0 + nsz],
                            ps[:, :nsz],
                            mybir.ActivationFunctionType.Identity,
                            bias=b_sb[:, o : o + 1],
                        )
                nc.sync.dma_start(
                    out_t[:, cb : cb + S].rearrange("(c p) s -> p c s", p=P), ob[:]
                )

            prev = None
            for b in range(NB):
                cb = b * S
                if b == 0:
                    xb = xb0
                else:
                    xb = xpool.tile([P, CH, S], BF16, tag="xb")
                    nc.sync.dma_start(
                        xb[:],
                        x_t[:, cb : cb + S].rearrange("(c p) s -> p c s", p=P),
                    )

                qT = qkpool.tile([P, CH, S], BF16, tag="qT")
                kT = qkpool.tile([P, CH, S], BF16, tag="kT")
                vtok = vpool.tile([P, G, C], BF16, tag="vtok")
                emit_qkv(xb, qT, kT, vtok)
                if b == 0:
                    # w_proj is first needed by out_proj(0), a batch later
                    for c in range(CH):
                        nc.sync.dma_start(wp_sb[:, c, :], wp_r[:, c, :])

                if prev is not None:
                    emit_out_proj(*prev)
                    prev = None

                # ---- attention ----
                yT = ypool.tile([P, CH, S], BF16, tag="yT")
                zt = zpool.tile([SC, G, NH], F32, tag="zt")
                ucls = zpool.tile([SC, G, NH], BF16, tag="ucls")
                sps = {}   # block t -> sp3 psum tile
                us = {}    # block t -> u3 sbuf tile (exp'd scores)
                uts = {}   # block t -> ut3 sbuf tile

                def stageA(t, qT=qT, kT=kT, sps=sps):
                    g, e = t // NBLK, t % NBLK
                    k0 = g * SC
                    pq = 0 if e < 2 else HD    # PE quadrant of this block
                    sp3 = ps_sc.tile([SC, BH, SC], F32, tag="sc")
                    qe = qen if g == 0 else qed
                    # one accumulation group per PSUM bank: mask opens it over
                    # the whole tile, per-head scores accumulate, last closes.
                    # All matmuls of the block use the same PE quadrant.
                    nc.tensor.matmul(
                        sp3[:, :, :],
                        lhsT=qe[pq : pq + HD, :],
                        rhs=ke5[pq : pq + HD, :],
                        start=True,
                        stop=False,
                    )
                    for hh in range(BH):
                        h = head_of(e, hh)
                        c = h // 2
                        nc.tensor.matmul(
                            sp3[:, hh, :],
                            lhsT=qT[pq : pq + HD, c, k0 : k0 + SC],
                            rhs=kT[pq : pq + HD, c, k0 : k0 + SC],
                            start=False,
                            stop=(hh == BH - 1),
                        )
                    sps[t] = sp3

                def stageB1(t, zt=zt, sps=sps, us=us):
                    g, e = t // NBLK, t % NBLK
                    sp3 = sps.pop(t)
                    u3 = upool.tile([SC, BH, SC], BF16, tag="u3")
                    nc.scalar.activation(
                        u3[:, :, :],
                        sp3[:, :, :],
                        mybir.ActivationFunctionType.Exp,
                        bias=biasq[:],
                    )
                    nc.vector.tensor_reduce(
                        zt[:, g, e * BH : (e + 1) * BH],
                        u3[:, :, :],
                        mybir.AxisListType.X,
                        mybir.AluOpType.add,
                    )
                    us[t] = u3

                def stageB2(g, rzb, ucls=ucls, us=us, uts=uts):
                    # normalize token-query rows in place (rzb row 0 is 1.0 so
                    # the cls row stays raw for the cross-slot cls softmax),
                    # then transpose per head
                    for e in range(NBLK):
                        t = g * NBLK + e
                        u3 = us.pop(t)
                        neng = norm_eng if e % 2 else nc.vector
                        neng.tensor_tensor(
                            u3[:, :, :],
                            u3[:, :, :],
                            rzb[:, e * BH : (e + 1) * BH].to_broadcast(
                                [SC, BH, SC]
                            ),
                            mybir.AluOpType.mult,
                        )
                        tp3 = ps_tp.tile([SC, BH, P], BF16, tag="tp")
                        for hh in range(BH):
                            nc.tensor.transpose(
                                tp3[:, hh, :SC], u3[:, hh, :], identb[:SC, :SC]
                            )
                        ut3 = upool.tile([SC, BH, SC], BF16, tag="ut3", bufs=5)
                        nc.vector.tensor_copy(ut3[:, :, :], tp3[:, :, :SC])
                        small_eng.tensor_copy(
                            ucls[:, g, e * BH : (e + 1) * BH], ut3[:, :, 0:1]
                        )
                        uts[t] = ut3

                def stageC(g, yT=yT, vtok=vtok, uts=uts):
                    k0 = g * SC
                    ut_blk = [uts.pop(g * NBLK + e) for e in range(NBLK)]
                    for c in range(CH):
                        yz = ps_yz.tile([P, SC], F32, tag="yz")
                        for hf in range(2):
                            h = 2 * c + hf
                            e, hh = blk_of(h)
                            nc.tensor.matmul(
                                yz[hf * HD : (hf + 1) * HD, 0:SC],
                                lhsT=vtok[:SC, g, h * HD : (h + 1) * HD],
                                rhs=ut_blk[e][:, hh, :],
                                start=True,
                                stop=True,
                            )
                        nc.vector.tensor_copy(
                            yT[:, c, k0 + 1 : k0 + SC], yz[:, 1:SC]
                        )

                NT = G * NBLK
                rzbs = {}
                for t in range(NT + 1):
                    if t < NT:
                        stageA(t)
                    if t >= 1:
                        stageB1(t - 1)
                        if (t - 1) % NBLK == NBLK - 1:
                            g = (t - 1) // NBLK
                            rzf = zpool.tile([SC, NH], F32, tag="rzf")
                            nc.vector.reciprocal(rzf[:], zt[:, g, :])
                            rzb = zpool.tile([SC, NH], BF16, tag="rzb")
                            small_eng.tensor_copy(rzb[:], rzf[:])
                            small_eng.memset(rzb[0:1, :], 1.0)
                            rzbs[g] = rzb
                            # B2/C of the previous slot: their inputs have had
                            # a full slot of slack, so PE never stalls on them
                            if g >= 1:
                                stageB2(g - 1, rzbs.pop(g - 1))
                                stageC(g - 1)
                stageB2(G - 1, rzbs.pop(G - 1))
                stageC(G - 1)

                # ---- cls finalize ----
                zcls = zpool.tile([1, NH], F32, tag="zcls")
                small_eng.tensor_copy(zcls[:], zt[0:1, 0, :])
                for g in range(1, G):
                    small_eng.tensor_tensor(
                        zcls[:], zcls[:], zt[0:1, g, :], mybir.AluOpType.add
                    )
                rzc = zpool.tile([1, NH], F32, tag="rzc")
                nc.vector.reciprocal(rzc[:], zcls[:])
                rzcp = ps_tp.tile([NH, 1], F32, tag="tp")
                nc.tensor.transpose(rzcp[:], rzc[:], identf[:1, :1])
                rzcT = zpool.tile([NH, 1], BF16, tag="rzcT")
                nc.vector.tensor_copy(rzcT[:], rzcp[:])
                for c in range(CH):
                    ycls = ps_yz.tile([P, SC], F32, tag="yz")
                    for g in range(G):
                        for hf in range(2):
                            h = 2 * c + hf
                            e, hh = blk_of(h)
                            nc.tensor.matmul(
                                ycls[hf * HD : (hf + 1) * HD, 0:1],
                                lhsT=vtok[:SC, g, h * HD : (h + 1) * HD],
                                rhs=ucls[:, g, e * BH + hh : e * BH + hh + 1],
                                start=(g == 0),
                                stop=(g == G - 1),
                            )
                    rzbc = ps_tp.tile([P, 1], F32, tag="tp")
                    nc.tensor.matmul(
                        rzbc[:],
                        lhsT=selc[:, c * P : (c + 1) * P],
                        rhs=rzcT[:],
                        start=True,
                        stop=True,
                    )
                    rzbc_sb = zpool.tile([P, 1], BF16, tag="rzbc")
                    nc.vector.tensor_copy(rzbc_sb[:], rzbc[:])
                    nc.vector.tensor_tensor(
                        yT[:, c, 0:1],
                        ycls[:, 0:1],
                        rzbc_sb[:],
                        mybir.AluOpType.mult,
                    )

                ob = opool.tile([P, CH, S], BF16, tag="ob")
                prev = (yT, ob, cb)

            emit_out_proj(*prev)

    nc.compile()
    return nc


_NC_CACHE = None
_LAST_IN_MAPS = None


def kernel(x, w_qkv, w_proj, b_proj):
    global _NC_CACHE, _LAST_IN_MAPS
    x = np.asarray(x)
    w_qkv = np.asarray(w_qkv)
    w_proj = np.asarray(w_proj)
    b_proj = np.asarray(b_proj)

    perm, valid = _perm_valid()
    qen, qed, ke5, biasq, selc = _consts()

    wq = np.array(w_qkv, np.float32, copy=True)
    wq[:, :C] *= 1.0 / np.sqrt(HD)
    wq = wq.astype(BFNP)
    wp = w_proj.astype(BFNP)
    b_pc = np.ascontiguousarray(b_proj.astype(np.float32).reshape(CH, P).T)

    in_maps = []
    for core in range(NCORES):
        xs = x[core * NB : (core + 1) * NB]          # (NB, 785, C)
        xp = xs[:, perm, :]                          # (NB, S, C)
        x_T = np.ascontiguousarray(
            xp.transpose(2, 0, 1).reshape(C, TT)
        ).astype(BFNP)
        in_maps.append(
            {
                "x_t": x_T,
                "w_qkv": wq,
                "w_proj": wp,
                "b_pc": b_pc,
                "qen": qen,
                "qed": qed,
                "ke5": ke5,
                "biasq": biasq,
                "selc": selc,
            }
        )

    if _NC_CACHE is None:
        _NC_CACHE = build_bass()
    nc = _NC_CACHE

    _LAST_IN_MAPS = in_maps

    res = run_bass_kernel_spmd(nc, in_maps, core_ids=list(range(NCORES)))

    out = np.zeros((B_TOTAL, N_TOK, C), np.float32)
    vperm = perm[valid]
    for core in range(NCORES):
        o_t = np.asarray(res.results[core]["out_t"]).astype(np.float32)
        op = o_t.reshape(C, NB, S).transpose(1, 2, 0)  # (NB, S, C)
        out[core * NB : (core + 1) * NB][:, vperm, :] = op[:, valid, :]
    return out


if __name__ == "__main__":
    rng = np.random.default_rng(0)
    x = rng.standard_normal((B_TOTAL, N_TOK, C)).astype(np.float32)
    w_qkv = (rng.standard_normal((C, 3 * C)) * 0.02).astype(np.float32)
    w_proj = (rng.standard_normal((C, C)) * 0.02).astype(np.float32)
    b_proj = np.zeros((C,), np.float32)
    y = kernel(x=x, w_qkv=w_qkv, w_proj=w_proj, b_proj=b_proj)
    print(np.abs(y).mean(), y.shape, y.dtype)


# revision 26
# speedup vs baseline: 1.3914x; 1.0188x over previous
"""AxialAttention (vertical, cls token, full cls attention) Trainium2 kernel.

Data-parallel over batch (32 batches -> 8 cores x 4 batches). Per core one
fused Bass/Tile program: qkv projection -> axial attention -> out projection.

Layout per batch: tokens regrouped into 7 slots of 113 columns
[cls, 4 rows x 28 tokens]; on-chip feature-major x_T (768, 791*4).

Attention is blocked per (slot g, head-block e) where each block holds 3
heads of equal parity (even heads live at partitions 0-63 of their feature
chunk, odd at 64-127), so every matmul in a block's PSUM accumulation group
uses the same PE quadrant (mixing tile positions within one bank group
faults real hardware). Per block:
  sp3[113 q, 3*113 k]: mask matmul (rank-5: +30 row-match terms, a cls-kill
  term for dup-cls keys) opens the bank group; 3 per-head score matmuls
  accumulate, last one closes. ONE batched exp (bias 0 for the cls query
  row, -30 for token rows) -> u3 bf16. DVE segmented tensor_reduce -> Z per
  (query, head). After a slot's 4 blocks: rz = 1/Z (cls row forced to 1),
  u3 token rows normalized in place, PE transposes per head into one PSUM
  tile, ONE DVE copy -> ut3. AV matmuls then need no transpose and yT
  writes are plain DVE copies. The cls query rides as column 0 of each
  block: its unnormalized weights are saved (ucls) and combined at batch
  end with the cross-slot Z sum into the cls output column.

Engines: PE matmuls; ACT exps + psum->sbuf copies (+bias); DVE reduces,
ut copies, yT writes; Pool(gpsimd) u3 normalize + small SBUF ops.
Output DMA'd as bf16, converted on host.
"""

import os

os.environ.setdefault("JAX_PLATFORMS", "axon")

import sys

if "/opt/trn_rl_repo" not in sys.path:
    sys.path.insert(0, "/opt/trn_rl_repo")

import numpy as np
import ml_dtypes

import concourse.bass as bass
import concourse.bacc as bacc
import concourse.mybir as mybir
import concourse.tile as tile
from concourse.bass_utils import run_bass_kernel_spmd
from concourse.masks import make_identity

P = 128
C = 768
CH = C // P            # 6 feature chunks
NH = 12
HD = 64
HH = 28                # image H = W
RG = 4                 # rows per slot
G = 7                  # slots per batch
W = RG * HH            # 112 token queries per slot
SC = W + 1             # 113 columns per slot (cls + tokens)
S = G * SC             # 791 columns per batch
NB = 4                 # batches per core
TT = NB * S            # 3164 columns per core
NCORES = 8
B_TOTAL = 32
N_TOK = 1 + HH * HH    # 785
MPEN = 30.0            # mask magnitude
KILL = -60.0           # dup-cls key kill (cls query, slots g>0)
NBLK = 4               # head blocks per slot (2 even-parity + 2 odd-parity)
BH = NH // NBLK        # heads per block = 3

F32 = mybir.dt.float32
BF16 = mybir.dt.bfloat16
BFNP = ml_dtypes.bfloat16

NORM_ENG = os.environ.get("BASSK_NORM_ENGINE", "pool")
SMALL_ENG = os.environ.get("BASSK_SMALL_ENGINE", "pool")


def head_of(e, hh):
    """block (e, hh) -> head index; blocks 0,1 = even heads, 2,3 = odd."""
    if e < 2:
        return 2 * (e * BH + hh)
    return 2 * ((e - 2) * BH + hh) + 1


def blk_of(h):
    """head -> (block e, lane hh)."""
    i = h // 2
    if h % 2 == 0:
        return i // BH, i % BH
    return 2 + i // BH, i % BH


def _perm_valid():
    """original-token index for each of the S slot-layout columns + validity."""
    perm = np.zeros(S, np.int64)
    valid = np.ones(S, np.bool_)
    for g in range(G):
        perm[g * SC] = 0
        if g > 0:
            valid[g * SC] = False
        for j in range(W):
            r = RG * g + j // HH   # row index (original column w)
            i = j % HH             # position in row (original row h)
            perm[g * SC + 1 + j] = 1 + i * HH + r
    return perm, valid


def _consts():
    rt = np.sqrt(MPEN)
    qe = np.zeros((5, SC), np.float32)
    ke1 = np.zeros((5, SC), np.float32)
    for j in range(W):
        qe[j // HH, 1 + j] = rt
        ke1[j // HH, 1 + j] = rt
    ke1[:RG, 0] = rt
    ke1[4, 0] = 1.0
    qen = np.zeros((P, SC), np.float32)
    qen[0:5] = qe
    qen[64:69] = qe
    qed = qen.copy()
    qed[4, 0] = KILL
    qed[68, 0] = KILL
    ke5 = np.zeros((P, BH * SC), np.float32)
    ke5[0:5] = np.tile(ke1, (1, BH))
    ke5[64:69] = ke5[0:5]
    biasq = np.full((SC, 1), -MPEN, np.float32)
    biasq[0, 0] = 0.0
    selc = np.zeros((NH, CH * P), np.float32)
    for h in range(NH):
        c, hf = h // 2, h % 2
        selc[h, c * P + hf * HD : c * P + (hf + 1) * HD] = 1.0
    return (
        qen.astype(BFNP),
        qed.astype(BFNP),
        ke5.astype(BFNP),
        biasq,
        selc.astype(BFNP),
    )


def build_bass():
    nc = bacc.Bacc(None, target_bir_lowering=False, debug=True)

    x_t = nc.declare_dram_parameter("x_t", [C, TT], BF16, isOutput=False)
    w_qkv = nc.declare_dram_parameter("w_qkv", [C, 3 * C], BF16, isOutput=False)
    w_proj = nc.declare_dram_parameter("w_proj", [C, C], BF16, isOutput=False)
    b_pc = nc.declare_dram_parameter("b_pc", [P, CH], F32, isOutput=False)
    qen_d = nc.declare_dram_parameter("qen", [P, SC], BF16, isOutput=False)
    qed_d = nc.declare_dram_parameter("qed", [P, SC], BF16, isOutput=False)
    ke5_d = nc.declare_dram_parameter("ke5", [P, BH * SC], BF16, isOutput=False)
    biasq_d = nc.declare_dram_parameter("biasq", [SC, 1], F32, isOutput=False)
    selc_d = nc.declare_dram_parameter("selc", [NH, CH * P], BF16, isOutput=False)
    out_t = nc.declare_dram_parameter("out_t", [C, TT], BF16, isOutput=True)

    NSPLIT = [(0, 512), (512, S - 512)]          # moving-dim tiling of S
    VSPLIT = [(0, 512), (512, C - 512)]          # moving-dim tiling of C (v cols)

    with tile.TileContext(nc) as tc:
        with (
            tc.tile_pool(name="const", bufs=1) as cpool,
            tc.tile_pool(name="xb", bufs=2) as xpool,
            tc.tile_pool(name="qk", bufs=2) as qkpool,
            tc.tile_pool(name="vt", bufs=2) as vpool,
            tc.tile_pool(name="yt", bufs=2) as ypool,
            tc.tile_pool(name="ob", bufs=2) as opool,
            tc.tile_pool(name="us", bufs=9) as upool,
            tc.tile_pool(name="zs", bufs=2) as zpool,
            tc.tile_pool(name="ps_mm", bufs=2, space="PSUM") as ps_mm,
            tc.tile_pool(name="ps_sc", bufs=2, space="PSUM") as ps_sc,
            tc.tile_pool(name="ps_tp", bufs=2, space="PSUM") as ps_tp,
            tc.tile_pool(name="ps_yz", bufs=2, space="PSUM") as ps_yz,
        ):
            # ---- constants ----
            wq_sb = cpool.tile([P, CH, 3 * C], BF16)
            wp_sb = cpool.tile([P, CH, C], BF16)
            wq_r = w_qkv.rearrange("(c p) o -> p c o", p=P)
            wp_r = w_proj.rearrange("(c p) o -> p c o", p=P)
            xb0 = xpool.tile([P, CH, S], BF16, tag="xb")
            x0_r = x_t[:, 0:S].rearrange("(c p) s -> p c s", p=P)
            nc.sync.dma_start(xb0[:, :, 0:512], x0_r[:, :, 0:512])
            for (w0, w1) in [(0, 256), (256, 768), (768, 1536), (1536, 2304)]:
                for c in range(CH):
                    nc.sync.dma_start(wq_sb[:, c, w0:w1], wq_r[:, c, w0:w1])
                if w0 == 0:
                    nc.sync.dma_start(xb0[:, :, 512:S], x0_r[:, :, 512:S])
            b_sb = cpool.tile([P, CH], F32)
            nc.sync.dma_start(b_sb[:], b_pc[:])
            qen = cpool.tile([P, SC], BF16)
            nc.sync.dma_start(qen[:], qen_d[:])
            qed = cpool.tile([P, SC], BF16)
            nc.sync.dma_start(qed[:], qed_d[:])
            ke5 = cpool.tile([P, BH * SC], BF16)
            nc.sync.dma_start(ke5[:], ke5_d[:])
            biasq = cpool.tile([SC, 1], F32)
            nc.sync.dma_start(biasq[:], biasq_d[:])
            selc = cpool.tile([NH, CH * P], BF16)
            nc.sync.dma_start(selc[:], selc_d[:])
            identb = cpool.tile([P, P], BF16)
            make_identity(nc, identb[:])
            identf = cpool.tile([P, P], F32)
            make_identity(nc, identf[:])

            norm_eng = {"pool": nc.gpsimd, "vector": nc.vector}[NORM_ENG]
            small_eng = {"pool": nc.gpsimd, "vector": nc.vector}[SMALL_ENG]

            def emit_qkv(xb, qT, kT, vtok):
                # q and k: feature-major output [feat chunk, token]
                for o in range(2 * CH):
                    dst = qT if o < CH else kT
                    oc = o % CH
                    for (n0, nsz) in NSPLIT:
                        ps = ps_mm.tile([P, 512], F32, tag="mm")
                        for c in range(CH):
                            nc.tensor.matmul(
                                ps[:, :nsz],
                                lhsT=wq_sb[:, c, o * P : (o + 1) * P],
                                rhs=xb[:, c, n0 : n0 + nsz],
                                start=(c == 0),
                                stop=(c == CH - 1),
                            )
                        nc.scalar.copy(dst[:, oc, n0 : n0 + nsz], ps[:, :nsz])
                # v: token-major per slot [113 tokens, C]
                for g in range(G):
                    for (v0, vsz) in VSPLIT:
                        ps = ps_mm.tile([P, 512], F32, tag="mm")
                        for c in range(CH):
                            nc.tensor.matmul(
                                ps[:SC, :vsz],
                                lhsT=xb[:, c, g * SC : (g + 1) * SC],
                                rhs=wq_sb[:, c, 2 * C + v0 : 2 * C + v0 + vsz],
                                start=(c == 0),
                                stop=(c == CH - 1),
                            )
                        nc.scalar.copy(vtok[:SC, g, v0 : v0 + vsz], ps[:SC, :vsz])

            def emit_out_proj(yT, ob, cb):
                for o in range(CH):
                    for (n0, nsz) in NSPLIT:
                        ps = ps_mm.tile([P, 512], F32, tag="mm")
                        for c in range(CH):
                            nc.tensor.matmul(
                                ps[:, :nsz],
                                lhsT=wp_sb[:, c, o * P : (o + 1) * P],
                                rhs=yT[:, c, n0 : n0 + nsz],
                                start=(c == 0),
                                stop=(c == CH - 1),
                            )
                        nc.scalar.activation(
                            ob[:, o, n0 : n0 + nsz],
                            ps[:, :nsz],
                            mybir.ActivationFunctionType.Identity,
                            bias=b_sb[:, o : o + 1],
                        )
                nc.sync.dma_start(
                    out_t[:, cb : cb + S].rearrange("(c p) s -> p c s", p=P), ob[:]
                )

            def emit_cls(zt, ucls, yT, vtok):
                zcls = zpool.tile([1, NH], F32, tag="zcls")
                small_eng.tensor_copy(zcls[:], zt[0:1, 0, :])
                for g in range(1, G):
                    small_eng.tensor_tensor(
                        zcls[:], zcls[:], zt[0:1, g, :], mybir.AluOpType.add
                    )
                rzc = zpool.tile([1, NH], F32, tag="rzc")
                nc.vector.reciprocal(rzc[:], zcls[:])
                rzcp = ps_tp.tile([NH, 1], F32, tag="tp")
                nc.tensor.transpose(rzcp[:], rzc[:], identf[:1, :1])
                rzcT = zpool.tile([NH, 1], BF16, tag="rzcT")
                nc.vector.tensor_copy(rzcT[:], rzcp[:])
                for c in range(CH):
                    ycls = ps_yz.tile([P, SC], F32, tag="yz")
                    for g in range(G):
                        for hf in range(2):
                            h = 2 * c + hf
                            e, hh = blk_of(h)
                            nc.tensor.matmul(
                                ycls[hf * HD : (hf + 1) * HD, 0:1],
                                lhsT=vtok[:SC, g, h * HD : (h + 1) * HD],
                                rhs=ucls[:, g, e * BH + hh : e * BH + hh + 1],
                                start=(g == 0),
                                stop=(g == G - 1),
                            )
                    rzbc = ps_tp.tile([P, 1], F32, tag="tp")
                    nc.tensor.matmul(
                        rzbc[:],
                        lhsT=selc[:, c * P : (c + 1) * P],
                        rhs=rzcT[:],
                        start=True,
                        stop=True,
                    )
                    rzbc_sb = zpool.tile([P, 1], BF16, tag="rzbc")
                    nc.vector.tensor_copy(rzbc_sb[:], rzbc[:])
                    nc.vector.tensor_tensor(
                        yT[:, c, 0:1],
                        ycls[:, 0:1],
                        rzbc_sb[:],
                        mybir.AluOpType.mult,
                    )

            prev = None
            pending_tail = None
            for b in range(NB):
                cb = b * S
                if b == 0:
                    xb = xb0
                else:
                    xb = xpool.tile([P, CH, S], BF16, tag="xb")
                    nc.sync.dma_start(
                        xb[:],
                        x_t[:, cb : cb + S].rearrange("(c p) s -> p c s", p=P),
                    )

                qT = qkpool.tile([P, CH, S], BF16, tag="qT")
                kT = qkpool.tile([P, CH, S], BF16, tag="kT")
                vtok = vpool.tile([P, G, C], BF16, tag="vtok")
                emit_qkv(xb, qT, kT, vtok)
                if b == 0:
                    # w_proj is first needed by out_proj(0), a batch later
                    for c in range(CH):
                        nc.sync.dma_start(wp_sb[:, c, :], wp_r[:, c, :])

                if pending_tail is not None:
                    pending_tail()
                    pending_tail = None
                if prev is not None:
                    emit_out_proj(*prev)
                    prev = None

                # ---- attention ----
                yT = ypool.tile([P, CH, S], BF16, tag="yT")
                zt = zpool.tile([SC, G, NH], F32, tag="zt")
                ucls = zpool.tile([SC, G, NH], BF16, tag="ucls")
                sps = {}   # block t -> sp3 psum tile
                us = {}    # block t -> u3 sbuf tile (exp'd scores)
                uts = {}   # block t -> ut3 sbuf tile

                def stageA(t, qT=qT, kT=kT, sps=sps):
                    g, e = t // NBLK, t % NBLK
                    k0 = g * SC
                    pq = 0 if e < 2 else HD    # PE quadrant of this block
                    sp3 = ps_sc.tile([SC, BH, SC], F32, tag="sc")
                    qe = qen if g == 0 else qed
                    # one accumulation group per PSUM bank: mask opens it over
                    # the whole tile, per-head scores accumulate, last closes.
                    # All matmuls of the block use the same PE quadrant.
                    nc.tensor.matmul(
                        sp3[:, :, :],
                        lhsT=qe[pq : pq + HD, :],
                        rhs=ke5[pq : pq + HD, :],
                        start=True,
                        stop=False,
                    )
                    for hh in range(BH):
                        h = head_of(e, hh)
                        c = h // 2
                        nc.tensor.matmul(
                            sp3[:, hh, :],
                            lhsT=qT[pq : pq + HD, c, k0 : k0 + SC],
                            rhs=kT[pq : pq + HD, c, k0 : k0 + SC],
                            start=False,
                            stop=(hh == BH - 1),
                        )
                    sps[t] = sp3

                def stageB1(t, zt=zt, sps=sps, us=us):
                    g, e = t // NBLK, t % NBLK
                    sp3 = sps.pop(t)
                    u3 = upool.tile([SC, BH, SC], BF16, tag="u3")
                    nc.scalar.activation(
                        u3[:, :, :],
                        sp3[:, :, :],
                        mybir.ActivationFunctionType.Exp,
                        bias=biasq[:],
                    )
                    nc.vector.tensor_reduce(
                        zt[:, g, e * BH : (e + 1) * BH],
                        u3[:, :, :],
                        mybir.AxisListType.X,
                        mybir.AluOpType.add,
                    )
                    us[t] = u3

                def stageB2(g, rzb, ucls=ucls, us=us, uts=uts):
                    # normalize token-query rows in place (rzb row 0 is 1.0 so
                    # the cls row stays raw for the cross-slot cls softmax),
                    # then transpose per head
                    for e in range(NBLK):
                        t = g * NBLK + e
                        u3 = us.pop(t)
                        neng = norm_eng if e % 2 else nc.vector
                        neng.tensor_tensor(
                            u3[:, :, :],
                            u3[:, :, :],
                            rzb[:, e * BH : (e + 1) * BH].to_broadcast(
                                [SC, BH, SC]
                            ),
                            mybir.AluOpType.mult,
                        )
                        tp3 = ps_tp.tile([SC, BH, P], BF16, tag="tp")
                        for hh in range(BH):
                            nc.tensor.transpose(
                                tp3[:, hh, :SC], u3[:, hh, :], identb[:SC, :SC]
                            )
                        ut3 = upool.tile([SC, BH, SC], BF16, tag="ut3", bufs=5)
                        nc.vector.tensor_copy(ut3[:, :, :], tp3[:, :, :SC])
                        small_eng.tensor_copy(
                            ucls[:, g, e * BH : (e + 1) * BH], ut3[:, :, 0:1]
                        )
                        uts[t] = ut3

                def stageC(g, yT=yT, vtok=vtok, uts=uts):
                    k0 = g * SC
                    ut_blk = [uts.pop(g * NBLK + e) for e in range(NBLK)]
                    for c in range(CH):
                        yz = ps_yz.tile([P, SC], F32, tag="yz")
                        for hf in range(2):
                            h = 2 * c + hf
                            e, hh = blk_of(h)
                            nc.tensor.matmul(
                                yz[hf * HD : (hf + 1) * HD, 0:SC],
                                lhsT=vtok[:SC, g, h * HD : (h + 1) * HD],
                                rhs=ut_blk[e][:, hh, :],
                                start=True,
                                stop=True,
                            )
                        nc.vector.tensor_copy(
                            yT[:, c, k0 + 1 : k0 + SC], yz[:, 1:SC]
                        )

                NT = G * NBLK
                rzbs = {}
                for t in range(NT + 1):
                    if t < NT:
                        stageA(t)
                    if t >= 1:
                        stageB1(t - 1)
                        if (t - 1) % NBLK == NBLK - 1:
                            g = (t - 1) // NBLK
                            rzf = zpool.tile([SC, NH], F32, tag="rzf")
                            nc.vector.reciprocal(rzf[:], zt[:, g, :])
                            rzb = zpool.tile([SC, NH], BF16, tag="rzb")
                            small_eng.tensor_copy(rzb[:], rzf[:])
                            small_eng.memset(rzb[0:1, :], 1.0)
                            rzbs[g] = rzb
                            # B2/C of the previous slot: their inputs have had
                            # a full slot of slack, so PE never stalls on them
                            if g >= 1:
                                stageB2(g - 1, rzbs.pop(g - 1))
                                stageC(g - 1)

                def finish_tail(rzbs=rzbs, stageB2=stageB2, stageC=stageC,
                                zt=zt, ucls=ucls, yT=yT, vtok=vtok):
                    stageB2(G - 1, rzbs.pop(G - 1))
                    stageC(G - 1)
                    emit_cls(zt, ucls, yT, vtok)

                pending_tail = finish_tail

                ob = opool.tile([P, CH, S], BF16, tag="ob")
                prev = (yT, ob, cb)

            pending_tail()
            emit_out_proj(*prev)

    nc.compile()
    return nc


_NC_CACHE = None
_LAST_IN_MAPS = None


def kernel(x, w_qkv, w_proj, b_proj):
    global _NC_CACHE, _LAST_IN_MAPS
    x = np.asarray(x)
    w_qkv = np.asarray(w_qkv)
    w_proj = np.asarray(w_proj)
    b_proj = np.asarray(b_proj)

    perm, valid = _perm_valid()
    qen, qed, ke5, biasq, selc = _consts()

    wq = np.array(w_qkv, np.float32, copy=True)
    wq[:, :C] *= 1.0 / np.sqrt(HD)
    wq = wq.astype(BFNP)
    wp = w_proj.astype(BFNP)
    b_pc = np.ascontiguousarray(b_proj.astype(np.float32).reshape(CH, P).T)

    in_maps = []
    for core in range(NCORES):
        xs = x[core * NB : (core + 1) * NB]          # (NB, 785, C)
        xp = xs[:, perm, :]                          # (NB, S, C)
        x_T = np.ascontiguousarray(
            xp.transpose(2, 0, 1).reshape(C, TT)
        ).astype(BFNP)
        in_maps.append(
            {
                "x_t": x_T,
                "w_qkv": wq,
                "w_proj": wp,
                "b_pc": b_pc,
                "qen": qen,
                "qed": qed,
                "ke5": ke5,
                "biasq": biasq,
                "selc": selc,
            }
        )

    if _NC_CACHE is None:
        _NC_CACHE = build_bass()
    nc = _NC_CACHE

    _LAST_IN_MAPS = in_maps

    res = run_bass_kernel_spmd(nc, in_maps, core_ids=list(range(NCORES)))

    out = np.zeros((B_TOTAL, N_TOK, C), np.float32)
    vperm = perm[valid]
    for core in range(NCORES):
        o_t = np.asarray(res.results[core]["out_t"]).astype(np.float32)
        op = o_t.reshape(C, NB, S).transpose(1, 2, 0)  # (NB, S, C)
        out[core * NB : (core + 1) * NB][:, vperm, :] = op[:, valid, :]
    return out


if __name__ == "__main__":
    rng = np.random.default_rng(0)
    x = rng.standard_normal((B_TOTAL, N_TOK, C)).astype(np.float32)
    w_qkv = (rng.standard_normal((C, 3 * C)) * 0.02).astype(np.float32)
    w_proj = (rng.standard_normal((C, C)) * 0.02).astype(np.float32)
    b_proj = np.zeros((C,), np.float32)
    y = kernel(x=x, w_qkv=w_qkv, w_proj=w_proj, b_proj=b_proj)
    print(np.abs(y).mean(), y.shape, y.dtype)


# revision 28
# speedup vs baseline: 1.4650x; 1.0529x over previous
"""AxialAttention (vertical, cls token, full cls attention) Trainium2 kernel.

Data-parallel over batch (32 batches -> 8 cores x 4 batches). Per core one
fused Bass/Tile program: qkv projection -> axial attention -> out projection.

Layout per batch: tokens regrouped into 7 slots of 113 columns
[cls, 4 rows x 28 tokens]; on-chip feature-major x_T (768, 791*4).

Attention is blocked per (slot g, head-block e) where each block holds 3
heads of equal parity (even heads live at partitions 0-63 of their feature
chunk, odd at 64-127), so every matmul in a block's PSUM accumulation group
uses the same PE quadrant (mixing tile positions within one bank group
faults real hardware). Per block:
  sp3[113 q, 3*113 k]: mask matmul (rank-5: +30 row-match terms, a cls-kill
  term for dup-cls keys) opens the bank group; 3 per-head score matmuls
  accumulate, last one closes. ONE batched exp (bias 0 for the cls query
  row, -30 for token rows) -> u3 bf16. DVE segmented tensor_reduce -> Z per
  (query, head). After a slot's 4 blocks: rz = 1/Z (cls row forced to 1),
  u3 token rows normalized in place, PE transposes per head into one PSUM
  tile, ONE DVE copy -> ut3. AV matmuls then need no transpose and yT
  writes are plain DVE copies. The cls query rides as column 0 of each
  block: its unnormalized weights are saved (ucls) and combined at batch
  end with the cross-slot Z sum into the cls output column.

Engines: PE matmuls; ACT exps + psum->sbuf copies (+bias); DVE reduces,
ut copies, yT writes; Pool(gpsimd) u3 normalize + small SBUF ops.
Output DMA'd as bf16, converted on host.
"""

import os

os.environ.setdefault("JAX_PLATFORMS", "axon")

import sys

if "/opt/trn_rl_repo" not in sys.path:
    sys.path.insert(0, "/opt/trn_rl_repo")

import numpy as np
import ml_dtypes

import concourse.bass as bass
import concourse.bacc as bacc
import concourse.mybir as mybir
import concourse.tile as tile
from concourse.bass_utils import run_bass_kernel_spmd
from concourse.masks import make_identity

P = 128
C = 768
CH = C // P            # 6 feature chunks
NH = 12
HD = 64
HH = 28                # image H = W
RG = 4                 # rows per slot
G = 7                  # slots per batch
W = RG * HH            # 112 token queries per slot
SC = W + 1             # 113 columns per slot (cls + tokens)
S = G * SC             # 791 columns per batch
NB = 4                 # batches per core
TT = NB * S            # 3164 columns per core
NCORES = 8
B_TOTAL = 32
N_TOK = 1 + HH * HH    # 785
MPEN = 30.0            # mask magnitude
KILL = -60.0           # dup-cls key kill (cls query, slots g>0)
NBLK = 4               # head blocks per slot (2 even-parity + 2 odd-parity)
BH = NH // NBLK        # heads per block = 3

F32 = mybir.dt.float32
BF16 = mybir.dt.bfloat16
BFNP = ml_dtypes.bfloat16

NORM_ENG = os.environ.get("BASSK_NORM_ENGINE", "pool")
SMALL_ENG = os.environ.get("BASSK_SMALL_ENGINE", "pool")


def head_of(e, hh):
    """block (e, hh) -> head index; blocks 0,1 = even heads, 2,3 = odd."""
    if e < 2:
        return 2 * (e * BH + hh)
    return 2 * ((e - 2) * BH + hh) + 1


def blk_of(h):
    """head -> (block e, lane hh)."""
    i = h // 2
    if h % 2 == 0:
        return i // BH, i % BH
    return 2 + i // BH, i % BH


def _perm_valid():
    """original-token index for each of the S slot-layout columns + validity."""
    perm = np.zeros(S, np.int64)
    valid = np.ones(S, np.bool_)
    for g in range(G):
        perm[g * SC] = 0
        if g > 0:
            valid[g * SC] = False
        for j in range(W):
            r = RG * g + j // HH   # row index (original column w)
            i = j % HH             # position in row (original row h)
            perm[g * SC + 1 + j] = 1 + i * HH + r
    return perm, valid


def _consts():
    rt = np.sqrt(MPEN)
    qe = np.zeros((5, SC), np.float32)
    ke1 = np.zeros((5, SC), np.float32)
    for j in range(W):
        qe[j // HH, 1 + j] = rt
        ke1[j // HH, 1 + j] = rt
    ke1[:RG, 0] = rt
    ke1[4, 0] = 1.0
    qen = np.zeros((P, SC), np.float32)
    qen[0:5] = qe
    qen[64:69] = qe
    qed = qen.copy()
    qed[4, 0] = KILL
    qed[68, 0] = KILL
    ke5 = np.zeros((P, BH * SC), np.float32)
    ke5[0:5] = np.tile(ke1, (1, BH))
    ke5[64:69] = ke5[0:5]
    biasq = np.full((SC, 1), -MPEN, np.float32)
    biasq[0, 0] = 0.0
    selc = np.zeros((NH, CH * P), np.float32)
    for h in range(NH):
        c, hf = h // 2, h % 2
        selc[h, c * P + hf * HD : c * P + (hf + 1) * HD] = 1.0
    return (
        qen.astype(BFNP),
        qed.astype(BFNP),
        ke5.astype(BFNP),
        biasq,
        selc.astype(BFNP),
    )


def build_bass():
    nc = bacc.Bacc(None, target_bir_lowering=False, debug=True)

    x_t = nc.declare_dram_parameter("x_t", [C, TT], BF16, isOutput=False)
    w_qkv = nc.declare_dram_parameter("w_qkv", [C, 3 * C], BF16, isOutput=False)
    w_proj = nc.declare_dram_parameter("w_proj", [C, C], BF16, isOutput=False)
    b_pc = nc.declare_dram_parameter("b_pc", [P, CH], F32, isOutput=False)
    qen_d = nc.declare_dram_parameter("qen", [P, SC], BF16, isOutput=False)
    qed_d = nc.declare_dram_parameter("qed", [P, SC], BF16, isOutput=False)
    ke5_d = nc.declare_dram_parameter("ke5", [P, BH * SC], BF16, isOutput=False)
    biasq_d = nc.declare_dram_parameter("biasq", [SC, 1], F32, isOutput=False)
    selc_d = nc.declare_dram_parameter("selc", [NH, CH * P], BF16, isOutput=False)
    out_t = nc.declare_dram_parameter("out_t", [C, TT], BF16, isOutput=True)

    NSPLIT = [(0, 512), (512, S - 512)]          # moving-dim tiling of S
    VSPLIT = [(0, 512), (512, C - 512)]          # moving-dim tiling of C (v cols)

    with tile.TileContext(nc) as tc:
        with (
            tc.tile_pool(name="const", bufs=1) as cpool,
            tc.tile_pool(name="xb", bufs=2) as xpool,
            tc.tile_pool(name="qk", bufs=2) as qkpool,
            tc.tile_pool(name="vt", bufs=2) as vpool,
            tc.tile_pool(name="yt", bufs=2) as ypool,
            tc.tile_pool(name="ob", bufs=2) as opool,
            tc.tile_pool(name="us", bufs=9) as upool,
            tc.tile_pool(name="zs", bufs=2) as zpool,
            tc.tile_pool(name="ps_mm", bufs=2, space="PSUM") as ps_mm,
            tc.tile_pool(name="ps_sc", bufs=2, space="PSUM") as ps_sc,
            tc.tile_pool(name="ps_tp", bufs=2, space="PSUM") as ps_tp,
            tc.tile_pool(name="ps_yz", bufs=2, space="PSUM") as ps_yz,
        ):
            # ---- constants ----
            wq_sb = cpool.tile([P, CH, 3 * C], BF16)
            wp_sb = cpool.tile([P, CH, C], BF16)
            wq_r = w_qkv.rearrange("(c p) o -> p c o", p=P)
            wp_r = w_proj.rearrange("(c p) o -> p c o", p=P)
            xb0 = xpool.tile([P, CH, S], BF16, tag="xb")
            x0_r = x_t[:, 0:S].rearrange("(c p) s -> p c s", p=P)
            nc.sync.dma_start(xb0[:, :, 0:512], x0_r[:, :, 0:512])
            for (w0, w1) in [(0, 256), (256, 768), (768, 1536), (1536, 2304)]:
                for c in range(CH):
                    nc.sync.dma_start(wq_sb[:, c, w0:w1], wq_r[:, c, w0:w1])
                if w0 == 0:
                    nc.sync.dma_start(xb0[:, :, 512:S], x0_r[:, :, 512:S])
            b_sb = cpool.tile([P, CH], F32)
            nc.sync.dma_start(b_sb[:], b_pc[:])
            qen = cpool.tile([P, SC], BF16)
            nc.sync.dma_start(qen[:], qen_d[:])
            qed = cpool.tile([P, SC], BF16)
            nc.sync.dma_start(qed[:], qed_d[:])
            ke5 = cpool.tile([P, BH * SC], BF16)
            nc.sync.dma_start(ke5[:], ke5_d[:])
            biasq = cpool.tile([SC, 1], F32)
            nc.sync.dma_start(biasq[:], biasq_d[:])
            selc = cpool.tile([NH, CH * P], BF16)
            nc.sync.dma_start(selc[:], selc_d[:])
            identb = cpool.tile([P, P], BF16)
            make_identity(nc, identb[:])
            identf = cpool.tile([P, P], F32)
            make_identity(nc, identf[:])

            norm_eng = {"pool": nc.gpsimd, "vector": nc.vector}[NORM_ENG]
            small_eng = {"pool": nc.gpsimd, "vector": nc.vector}[SMALL_ENG]

            def emit_qkv(xb, qT, kT, vtok):
                # q and k: feature-major output [feat chunk, token]
                for o in range(2 * CH):
                    dst = qT if o < CH else kT
                    oc = o % CH
                    for (n0, nsz) in NSPLIT:
                        ps = ps_mm.tile([P, 512], F32, tag="mm")
                        for c in range(CH):
                            nc.tensor.matmul(
                                ps[:, :nsz],
                                lhsT=wq_sb[:, c, o * P : (o + 1) * P],
                                rhs=xb[:, c, n0 : n0 + nsz],
                                start=(c == 0),
                                stop=(c == CH - 1),
                            )
                        nc.scalar.copy(dst[:, oc, n0 : n0 + nsz], ps[:, :nsz])
                # v: token-major per slot [113 tokens, C]
                for g in range(G):
                    for (v0, vsz) in VSPLIT:
                        ps = ps_mm.tile([P, 512], F32, tag="mm")
                        for c in range(CH):
                            nc.tensor.matmul(
                                ps[:SC, :vsz],
                                lhsT=xb[:, c, g * SC : (g + 1) * SC],
                                rhs=wq_sb[:, c, 2 * C + v0 : 2 * C + v0 + vsz],
                                start=(c == 0),
                                stop=(c == CH - 1),
                            )
                        nc.scalar.copy(vtok[:SC, g, v0 : v0 + vsz], ps[:SC, :vsz])

            def emit_out_proj_cols(yT, ob, cb, lo, wd):
                for o in range(CH):
                    ps = ps_mm.tile([P, 512], F32, tag="mm")
                    for c in range(CH):
                        nc.tensor.matmul(
                            ps[:, :wd],
                            lhsT=wp_sb[:, c, o * P : (o + 1) * P],
                            rhs=yT[:, c, lo : lo + wd],
                            start=(c == 0),
                            stop=(c == CH - 1),
                        )
                    nc.scalar.activation(
                        ob[:, o, lo : lo + wd],
                        ps[:, :wd],
                        mybir.ActivationFunctionType.Identity,
                        bias=b_sb[:, o : o + 1],
                    )
                nc.sync.dma_start(
                    out_t[:, cb + lo : cb + lo + wd].rearrange(
                        "(c p) s -> p c s", p=P
                    ),
                    ob[:, :, lo : lo + wd],
                )

            def emit_out_proj(yT, ob, cb):
                for o in range(CH):
                    for (n0, nsz) in NSPLIT:
                        ps = ps_mm.tile([P, 512], F32, tag="mm")
                        for c in range(CH):
                            nc.tensor.matmul(
                                ps[:, :nsz],
                                lhsT=wp_sb[:, c, o * P : (o + 1) * P],
                                rhs=yT[:, c, n0 : n0 + nsz],
                                start=(c == 0),
                                stop=(c == CH - 1),
                            )
                        nc.scalar.activation(
                            ob[:, o, n0 : n0 + nsz],
                            ps[:, :nsz],
                            mybir.ActivationFunctionType.Identity,
                            bias=b_sb[:, o : o + 1],
                        )
                nc.sync.dma_start(
                    out_t[:, cb : cb + S].rearrange("(c p) s -> p c s", p=P), ob[:]
                )

            def emit_cls(zt, ucls, yT, vtok):
                zcls = zpool.tile([1, NH], F32, tag="zcls")
                small_eng.tensor_copy(zcls[:], zt[0:1, 0, :])
                for g in range(1, G):
                    small_eng.tensor_tensor(
                        zcls[:], zcls[:], zt[0:1, g, :], mybir.AluOpType.add
                    )
                rzc = zpool.tile([1, NH], F32, tag="rzc")
                nc.vector.reciprocal(rzc[:], zcls[:])
                rzcp = ps_tp.tile([NH, 1], F32, tag="tp")
                nc.tensor.transpose(rzcp[:], rzc[:], identf[:1, :1])
                rzcT = zpool.tile([NH, 1], BF16, tag="rzcT")
                nc.vector.tensor_copy(rzcT[:], rzcp[:])
                for c in range(CH):
                    ycls = ps_yz.tile([P, SC], F32, tag="yz")
                    for g in range(G):
                        for hf in range(2):
                            h = 2 * c + hf
                            e, hh = blk_of(h)
                            nc.tensor.matmul(
                                ycls[hf * HD : (hf + 1) * HD, 0:1],
                                lhsT=vtok[:SC, g, h * HD : (h + 1) * HD],
                                rhs=ucls[:, g, e * BH + hh : e * BH + hh + 1],
                                start=(g == 0),
                                stop=(g == G - 1),
                            )
                    rzbc = ps_tp.tile([P, 1], F32, tag="tp")
                    nc.tensor.matmul(
                        rzbc[:],
                        lhsT=selc[:, c * P : (c + 1) * P],
                        rhs=rzcT[:],
                        start=True,
                        stop=True,
                    )
                    rzbc_sb = zpool.tile([P, 1], BF16, tag="rzbc")
                    nc.vector.tensor_copy(rzbc_sb[:], rzbc[:])
                    nc.vector.tensor_tensor(
                        yT[:, c, 0:1],
                        ycls[:, 0:1],
                        rzbc_sb[:],
                        mybir.AluOpType.mult,
                    )

            prev = None
            pending_tail = None
            for b in range(NB):
                cb = b * S
                if b == 0:
                    xb = xb0
                else:
                    xb = xpool.tile([P, CH, S], BF16, tag="xb")
                    nc.sync.dma_start(
                        xb[:],
                        x_t[:, cb : cb + S].rearrange("(c p) s -> p c s", p=P),
                    )

                qT = qkpool.tile([P, CH, S], BF16, tag="qT")
                kT = qkpool.tile([P, CH, S], BF16, tag="kT")
                vtok = vpool.tile([P, G, C], BF16, tag="vtok")
                emit_qkv(xb, qT, kT, vtok)
                if b == 0:
                    # w_proj is first needed by out_proj(0), a batch later
                    for c in range(CH):
                        nc.sync.dma_start(wp_sb[:, c, :], wp_r[:, c, :])

                if pending_tail is not None:
                    pending_tail()
                    pending_tail = None
                if prev is not None:
                    emit_out_proj(*prev)
                    prev = None

                # ---- attention ----
                if b == NB - 1:
                    ob_last = opool.tile([P, CH, S], BF16, tag="ob", name="ob")
                else:
                    ob_last = None
                yT = ypool.tile([P, CH, S], BF16, tag="yT")
                zt = zpool.tile([SC, G, NH], F32, tag="zt")
                ucls = zpool.tile([SC, G, NH], BF16, tag="ucls")
                sps = {}   # block t -> sp3 psum tile
                us = {}    # block t -> u3 sbuf tile (exp'd scores)
                uts = {}   # block t -> ut3 sbuf tile

                def stageA(t, qT=qT, kT=kT, sps=sps):
                    g, e = t // NBLK, t % NBLK
                    k0 = g * SC
                    pq = 0 if e < 2 else HD    # PE quadrant of this block
                    sp3 = ps_sc.tile([SC, BH, SC], F32, tag="sc")
                    qe = qen if g == 0 else qed
                    # one accumulation group per PSUM bank: mask opens it over
                    # the whole tile, per-head scores accumulate, last closes.
                    # All matmuls of the block use the same PE quadrant.
                    nc.tensor.matmul(
                        sp3[:, :, :],
                        lhsT=qe[pq : pq + HD, :],
                        rhs=ke5[pq : pq + HD, :],
                        start=True,
                        stop=False,
                    )
                    for hh in range(BH):
                        h = head_of(e, hh)
                        c = h // 2
                        nc.tensor.matmul(
                            sp3[:, hh, :],
                            lhsT=qT[pq : pq + HD, c, k0 : k0 + SC],
                            rhs=kT[pq : pq + HD, c, k0 : k0 + SC],
                            start=False,
                            stop=(hh == BH - 1),
                        )
                    sps[t] = sp3

                def stageB1(t, zt=zt, sps=sps, us=us):
                    g, e = t // NBLK, t % NBLK
                    sp3 = sps.pop(t)
                    u3 = upool.tile([SC, BH, SC], BF16, tag="u3")
                    nc.scalar.activation(
                        u3[:, :, :],
                        sp3[:, :, :],
                        mybir.ActivationFunctionType.Exp,
                        bias=biasq[:],
                    )
                    nc.vector.tensor_reduce(
                        zt[:, g, e * BH : (e + 1) * BH],
                        u3[:, :, :],
                        mybir.AxisListType.X,
                        mybir.AluOpType.add,
                    )
                    us[t] = u3

                def stageB2(g, rzb, ucls=ucls, us=us, uts=uts):
                    # normalize token-query rows in place (rzb row 0 is 1.0 so
                    # the cls row stays raw for the cross-slot cls softmax),
                    # then transpose per head
                    for e in range(NBLK):
                        t = g * NBLK + e
                        u3 = us.pop(t)
                        neng = norm_eng if e % 2 else nc.vector
                        neng.tensor_tensor(
                            u3[:, :, :],
                            u3[:, :, :],
                            rzb[:, e * BH : (e + 1) * BH].to_broadcast(
                                [SC, BH, SC]
                            ),
                            mybir.AluOpType.mult,
                        )
                        tp3 = ps_tp.tile([SC, BH, P], BF16, tag="tp")
                        for hh in range(BH):
                            nc.tensor.transpose(
                                tp3[:, hh, :SC], u3[:, hh, :], identb[:SC, :SC]
                            )
                        ut3 = upool.tile([SC, BH, SC], BF16, tag="ut3", bufs=5)
                        if e % 2:
                            nc.scalar.copy(ut3[:, :, :], tp3[:, :, :SC])
                        else:
                            nc.vector.tensor_copy(ut3[:, :, :], tp3[:, :, :SC])
                        small_eng.tensor_copy(
                            ucls[:, g, e * BH : (e + 1) * BH], ut3[:, :, 0:1]
                        )
                        uts[t] = ut3

                def stageC(g, yT=yT, vtok=vtok, uts=uts):
                    k0 = g * SC
                    ut_blk = [uts.pop(g * NBLK + e) for e in range(NBLK)]
                    for c in range(CH):
                        yz = ps_yz.tile([P, SC], F32, tag="yz")
                        for hf in range(2):
                            h = 2 * c + hf
                            e, hh = blk_of(h)
                            nc.tensor.matmul(
                                yz[hf * HD : (hf + 1) * HD, 0:SC],
                                lhsT=vtok[:SC, g, h * HD : (h + 1) * HD],
                                rhs=ut_blk[e][:, hh, :],
                                start=True,
                                stop=True,
                            )
                        nc.vector.tensor_copy(
                            yT[:, c, k0 + 1 : k0 + SC], yz[:, 1:SC]
                        )

                NT = G * NBLK
                rzbs = {}
                for t in range(NT + 1):
                    if t < NT:
                        stageA(t)
                    if t >= 1:
                        stageB1(t - 1)
                        if (t - 1) % NBLK == NBLK - 1:
                            g = (t - 1) // NBLK
                            rzf = zpool.tile([SC, NH], F32, tag="rzf")
                            nc.vector.reciprocal(rzf[:], zt[:, g, :])
                            rzb = zpool.tile([SC, NH], BF16, tag="rzb")
                            small_eng.tensor_copy(rzb[:], rzf[:])
                            small_eng.memset(rzb[0:1, :], 1.0)
                            rzbs[g] = rzb
                            # B2/C of the previous slot: their inputs have had
                            # a full slot of slack, so PE never stalls on them
                            if g >= 1:
                                stageB2(g - 1, rzbs.pop(g - 1))
                                stageC(g - 1)
                                if b == NB - 1:
                                    # drain: project finished slots immediately
                                    k0 = (g - 1) * SC
                                    lo = k0 + (1 if g - 1 == 0 else 0)
                                    emit_out_proj_cols(
                                        yT, ob_last, cb, lo, k0 + SC - lo
                                    )

                def finish_tail(rzbs=rzbs, stageB2=stageB2, stageC=stageC,
                                zt=zt, ucls=ucls, yT=yT, vtok=vtok,
                                ob_last=ob_last, cb=cb, b=b):
                    stageB2(G - 1, rzbs.pop(G - 1))
                    stageC(G - 1)
                    if b == NB - 1:
                        k0 = (G - 1) * SC
                        emit_out_proj_cols(yT, ob_last, cb, k0, SC)
                    emit_cls(zt, ucls, yT, vtok)
                    if b == NB - 1:
                        emit_out_proj_cols(yT, ob_last, cb, 0, 1)

                pending_tail = finish_tail

                if b < NB - 1:
                    ob = opool.tile([P, CH, S], BF16, tag="ob")
                    prev = (yT, ob, cb)

            pending_tail()

    nc.compile()
    return nc


_NC_CACHE = None
_LAST_IN_MAPS = None


def kernel(x, w_qkv, w_proj, b_proj):
    global _NC_CACHE, _LAST_IN_MAPS
    x = np.asarray(x)
    w_qkv = np.asarray(w_qkv)
    w_proj = np.asarray(w_proj)
    b_proj = np.asarray(b_proj)

    perm, valid = _perm_valid()
    qen, qed, ke5, biasq, selc = _consts()

    wq = np.array(w_qkv, np.float32, copy=True)
    wq[:, :C] *= 1.0 / np.sqrt(HD)
    wq = wq.astype(BFNP)
    wp = w_proj.astype(BFNP)
    b_pc = np.ascontiguousarray(b_proj.astype(np.float32).reshape(CH, P).T)

    in_maps = []
    for core in range(NCORES):
        xs = x[core * NB : (core + 1) * NB]          # (NB, 785, C)
        xp = xs[:, perm, :]                          # (NB, S, C)
        x_T = np.ascontiguousarray(
            xp.transpose(2, 0, 1).reshape(C, TT)
        ).astype(BFNP)
        in_maps.append(
            {
                "x_t": x_T,
                "w_qkv": wq,
                "w_proj": wp,
                "b_pc": b_pc,
                "qen": qen,
                "qed": qed,
                "ke5": ke5,
                "biasq": biasq,
                "selc": selc,
            }
        )

    if _NC_CACHE is None:
        _NC_CACHE = build_bass()
    nc = _NC_CACHE

    _LAST_IN_MAPS = in_maps

    res = run_bass_kernel_spmd(nc, in_maps, core_ids=list(range(NCORES)))

    out = np.zeros((B_TOTAL, N_TOK, C), np.float32)
    vperm = perm[valid]
    for core in range(NCORES):
        o_t = np.asarray(res.results[core]["out_t"]).astype(np.float32)
        op = o_t.reshape(C, NB, S).transpose(1, 2, 0)  # (NB, S, C)
        out[core * NB : (core + 1) * NB][:, vperm, :] = op[:, valid, :]
    return out


if __name__ == "__main__":
    rng = np.random.default_rng(0)
    x = rng.standard_normal((B_TOTAL, N_TOK, C)).astype(np.float32)
    w_qkv = (rng.standard_normal((C, 3 * C)) * 0.02).astype(np.float32)
    w_proj = (rng.standard_normal((C, C)) * 0.02).astype(np.float32)
    b_proj = np.zeros((C,), np.float32)
    y = kernel(x=x, w_qkv=w_qkv, w_proj=w_proj, b_proj=b_proj)
    print(np.abs(y).mean(), y.shape, y.dtype)


# revision 33
# speedup vs baseline: 1.4753x; 1.0070x over previous
"""AxialAttention (vertical, cls token, full cls attention) Trainium2 kernel.

Data-parallel over batch (32 batches -> 8 cores x 4 batches). Per core one
fused Bass/Tile program: qkv projection -> axial attention -> out projection.

Layout per batch: tokens regrouped into 7 slots of 113 columns
[cls, 4 rows x 28 tokens]; on-chip feature-major x_T (768, 791*4).

Attention is blocked per (slot g, head-block e) where each block holds 3
heads of equal parity (even heads live at partitions 0-63 of their feature
chunk, odd at 64-127), so every matmul in a block's PSUM accumulation group
uses the same PE quadrant (mixing tile positions within one bank group
faults real hardware). Per block:
  sp3[113 q, 3*113 k]: mask matmul (rank-5: +30 row-match terms, a cls-kill
  term for dup-cls keys) opens the bank group; 3 per-head score matmuls
  accumulate, last one closes. ONE batched exp (bias 0 for the cls query
  row, -30 for token rows) -> u3 bf16. DVE segmented tensor_reduce -> Z per
  (query, head). After a slot's 4 blocks: rz = 1/Z (cls row forced to 1),
  u3 token rows normalized in place, PE transposes per head into one PSUM
  tile, ONE DVE copy -> ut3. AV matmuls then need no transpose and yT
  writes are plain DVE copies. The cls query rides as column 0 of each
  block: its unnormalized weights are saved (ucls) and combined at batch
  end with the cross-slot Z sum into the cls output column.

Engines: PE matmuls; ACT exps + psum->sbuf copies (+bias); DVE reduces,
ut copies, yT writes; Pool(gpsimd) u3 normalize + small SBUF ops.
Output DMA'd as bf16, converted on host.
"""

import os

os.environ.setdefault("JAX_PLATFORMS", "axon")

import sys

if "/opt/trn_rl_repo" not in sys.path:
    sys.path.insert(0, "/opt/trn_rl_repo")

import numpy as np
import ml_dtypes

import concourse.bass as bass
import concourse.bacc as bacc
import concourse.mybir as mybir
import concourse.tile as tile
from concourse.bass_utils import run_bass_kernel_spmd
from concourse.masks import make_identity

P = 128
C = 768
CH = C // P            # 6 feature chunks
NH = 12
HD = 64
HH = 28                # image H = W
RG = 4                 # rows per slot
G = 7                  # slots per batch
W = RG * HH            # 112 token queries per slot
SC = W + 1             # 113 columns per slot (cls + tokens)
S = G * SC             # 791 columns per batch
NB = 4                 # batches per core
TT = NB * S            # 3164 columns per core
NCORES = 8
B_TOTAL = 32
N_TOK = 1 + HH * HH    # 785
MPEN = 30.0            # mask magnitude
KILL = -60.0           # dup-cls key kill (cls query, slots g>0)
NBLK = 4               # head blocks per slot (2 even-parity + 2 odd-parity)
BH = NH // NBLK        # heads per block = 3

F32 = mybir.dt.float32
BF16 = mybir.dt.bfloat16
BFNP = ml_dtypes.bfloat16

NORM_ENG = os.environ.get("BASSK_NORM_ENGINE", "pool")
SMALL_ENG = os.environ.get("BASSK_SMALL_ENGINE", "pool")


def head_of(e, hh):
    """block (e, hh) -> head index; blocks 0,1 = even heads, 2,3 = odd."""
    if e < 2:
        return 2 * (e * BH + hh)
    return 2 * ((e - 2) * BH + hh) + 1


def blk_of(h):
    """head -> (block e, lane hh)."""
    i = h // 2
    if h % 2 == 0:
        return i // BH, i % BH
    return 2 + i // BH, i % BH


def _perm_valid():
    """original-token index for each of the S slot-layout columns + validity."""
    perm = np.zeros(S, np.int64)
    valid = np.ones(S, np.bool_)
    for g in range(G):
        perm[g * SC] = 0
        if g > 0:
            valid[g * SC] = False
        for j in range(W):
            r = RG * g + j // HH   # row index (original column w)
            i = j % HH             # position in row (original row h)
            perm[g * SC + 1 + j] = 1 + i * HH + r
    return perm, valid


def _consts():
    rt = np.sqrt(MPEN)
    qe = np.zeros((5, SC), np.float32)
    ke1 = np.zeros((5, SC), np.float32)
    for j in range(W):
        qe[j // HH, 1 + j] = rt
        ke1[j // HH, 1 + j] = rt
    ke1[:RG, 0] = rt
    ke1[4, 0] = 1.0
    qen = np.zeros((P, SC), np.float32)
    qen[0:5] = qe
    qen[64:69] = qe
    qed = qen.copy()
    qed[4, 0] = KILL
    qed[68, 0] = KILL
    ke5 = np.zeros((P, BH * SC), np.float32)
    ke5[0:5] = np.tile(ke1, (1, BH))
    ke5[64:69] = ke5[0:5]
    biasq = np.full((SC, 1), -MPEN, np.float32)
    biasq[0, 0] = 0.0
    selc = np.zeros((NH, CH * P), np.float32)
    for h in range(NH):
        c, hf = h // 2, h % 2
        selc[h, c * P + hf * HD : c * P + (hf + 1) * HD] = 1.0
    return (
        qen.astype(BFNP),
        qed.astype(BFNP),
        ke5.astype(BFNP),
        biasq,
        selc.astype(BFNP),
    )


def build_bass():
    nc = bacc.Bacc(None, target_bir_lowering=False, debug=True)

    x_t = nc.declare_dram_parameter("x_t", [C, TT], BF16, isOutput=False)
    w_qkv = nc.declare_dram_parameter("w_qkv", [C, 3 * C], BF16, isOutput=False)
    w_proj = nc.declare_dram_parameter("w_proj", [C, C], BF16, isOutput=False)
    b_pc = nc.declare_dram_parameter("b_pc", [P, CH], F32, isOutput=False)
    qen_d = nc.declare_dram_parameter("qen", [P, SC], BF16, isOutput=False)
    qed_d = nc.declare_dram_parameter("qed", [P, SC], BF16, isOutput=False)
    ke5_d = nc.declare_dram_parameter("ke5", [P, BH * SC], BF16, isOutput=False)
    biasq_d = nc.declare_dram_parameter("biasq", [SC, 1], F32, isOutput=False)
    selc_d = nc.declare_dram_parameter("selc", [NH, CH * P], BF16, isOutput=False)
    out_t = nc.declare_dram_parameter("out_t", [C, TT], BF16, isOutput=True)

    NSPLIT = [(0, 512), (512, S - 512)]          # moving-dim tiling of S
    VSPLIT = [(0, 512), (512, C - 512)]          # moving-dim tiling of C (v cols)

    with tile.TileContext(nc) as tc:
        with (
            tc.tile_pool(name="const", bufs=1) as cpool,
            tc.tile_pool(name="xb", bufs=2) as xpool,
            tc.tile_pool(name="qk", bufs=2) as qkpool,
            tc.tile_pool(name="vt", bufs=2) as vpool,
            tc.tile_pool(name="yt", bufs=2) as ypool,
            tc.tile_pool(name="ob", bufs=2) as opool,
            tc.tile_pool(name="us", bufs=9) as upool,
            tc.tile_pool(name="zs", bufs=2) as zpool,
            tc.tile_pool(name="ps_mm", bufs=2, space="PSUM") as ps_mm,
            tc.tile_pool(name="ps_sc", bufs=2, space="PSUM") as ps_sc,
            tc.tile_pool(name="ps_tp", bufs=2, space="PSUM") as ps_tp,
            tc.tile_pool(name="ps_yz", bufs=2, space="PSUM") as ps_yz,
        ):
            # ---- constants ----
            wq_sb = cpool.tile([P, CH, 3 * C], BF16)
            wp_sb = cpool.tile([P, CH, C], BF16)
            wq_r = w_qkv.rearrange("(c p) o -> p c o", p=P)
            wp_r = w_proj.rearrange("(c p) o -> p c o", p=P)
            xb0 = xpool.tile([P, CH, S], BF16, tag="xb")
            x0_r = x_t[:, 0:S].rearrange("(c p) s -> p c s", p=P)
            nc.sync.dma_start(xb0[:, :, 0:512], x0_r[:, :, 0:512])
            for (w0, w1) in [(0, 256), (256, 768), (768, 1536), (1536, 2304)]:
                for c in range(CH):
                    nc.sync.dma_start(wq_sb[:, c, w0:w1], wq_r[:, c, w0:w1])
                if w0 == 0:
                    nc.sync.dma_start(xb0[:, :, 512:S], x0_r[:, :, 512:S])
            b_sb = cpool.tile([P, CH], F32)
            nc.sync.dma_start(b_sb[:], b_pc[:])
            qen = cpool.tile([P, SC], BF16)
            nc.sync.dma_start(qen[:], qen_d[:])
            qed = cpool.tile([P, SC], BF16)
            nc.sync.dma_start(qed[:], qed_d[:])
            ke5 = cpool.tile([P, BH * SC], BF16)
            nc.sync.dma_start(ke5[:], ke5_d[:])
            biasq = cpool.tile([SC, 1], F32)
            nc.sync.dma_start(biasq[:], biasq_d[:])
            selc = cpool.tile([NH, CH * P], BF16)
            nc.sync.dma_start(selc[:], selc_d[:])
            identb = cpool.tile([P, P], BF16)
            make_identity(nc, identb[:])
            identf = cpool.tile([P, P], F32)
            make_identity(nc, identf[:])

            norm_eng = {"pool": nc.gpsimd, "vector": nc.vector}[NORM_ENG]
            small_eng = {"pool": nc.gpsimd, "vector": nc.vector}[SMALL_ENG]

            def emit_qkv(xb, qT, kT, vtok):
                def qkv_copy(dst, src):
                    nc.scalar.copy(dst, src)

                # q and k: feature-major output [feat chunk, token]
                for o in range(2 * CH):
                    dst = qT if o < CH else kT
                    oc = o % CH
                    for (n0, nsz) in NSPLIT:
                        ps = ps_mm.tile([P, 512], F32, tag="mm")
                        for c in range(CH):
                            nc.tensor.matmul(
                                ps[:, :nsz],
                                lhsT=wq_sb[:, c, o * P : (o + 1) * P],
                                rhs=xb[:, c, n0 : n0 + nsz],
                                start=(c == 0),
                                stop=(c == CH - 1),
                            )
                        qkv_copy(dst[:, oc, n0 : n0 + nsz], ps[:, :nsz])
                # v: token-major per slot [113 tokens, C]
                for g in range(G):
                    for (v0, vsz) in VSPLIT:
                        ps = ps_mm.tile([P, 512], F32, tag="mm")
                        for c in range(CH):
                            nc.tensor.matmul(
                                ps[:SC, :vsz],
                                lhsT=xb[:, c, g * SC : (g + 1) * SC],
                                rhs=wq_sb[:, c, 2 * C + v0 : 2 * C + v0 + vsz],
                                start=(c == 0),
                                stop=(c == CH - 1),
                            )
                        qkv_copy(vtok[:SC, g, v0 : v0 + vsz], ps[:SC, :vsz])

            def emit_out_proj_cols(yT, ob, cb, lo, wd):
                for o in range(CH):
                    ps = ps_mm.tile([P, 512], F32, tag="mm")
                    for c in range(CH):
                        nc.tensor.matmul(
                            ps[:, :wd],
                            lhsT=wp_sb[:, c, o * P : (o + 1) * P],
                            rhs=yT[:, c, lo : lo + wd],
                            start=(c == 0),
                            stop=(c == CH - 1),
                        )
                    nc.scalar.activation(
                        ob[:, o, lo : lo + wd],
                        ps[:, :wd],
                        mybir.ActivationFunctionType.Identity,
                        bias=b_sb[:, o : o + 1],
                    )
                nc.sync.dma_start(
                    out_t[:, cb + lo : cb + lo + wd].rearrange(
                        "(c p) s -> p c s", p=P
                    ),
                    ob[:, :, lo : lo + wd],
                )

            def emit_out_proj(yT, ob, cb):
                # high split first: col 0 (cls) is written last by emit_cls,
                # so give its chain more slack
                for o in range(CH):
                    for (n0, nsz) in reversed(NSPLIT):
                        ps = ps_mm.tile([P, 512], F32, tag="mm")
                        for c in range(CH):
                            nc.tensor.matmul(
                                ps[:, :nsz],
                                lhsT=wp_sb[:, c, o * P : (o + 1) * P],
                                rhs=yT[:, c, n0 : n0 + nsz],
                                start=(c == 0),
                                stop=(c == CH - 1),
                            )
                        nc.scalar.activation(
                            ob[:, o, n0 : n0 + nsz],
                            ps[:, :nsz],
                            mybir.ActivationFunctionType.Identity,
                            bias=b_sb[:, o : o + 1],
                        )
                nc.sync.dma_start(
                    out_t[:, cb : cb + S].rearrange("(c p) s -> p c s", p=P), ob[:]
                )

            def emit_cls(zt, ucls, yT, vtok):
                zcls = zpool.tile([1, NH], F32, tag="zcls")
                small_eng.tensor_copy(zcls[:], zt[0:1, 0, :])
                for g in range(1, G):
                    small_eng.tensor_tensor(
                        zcls[:], zcls[:], zt[0:1, g, :], mybir.AluOpType.add
                    )
                rzc = zpool.tile([1, NH], F32, tag="rzc")
                nc.vector.reciprocal(rzc[:], zcls[:])
                rzcp = ps_tp.tile([NH, 1], F32, tag="tp")
                nc.tensor.transpose(rzcp[:], rzc[:], identf[:1, :1])
                rzcT = zpool.tile([NH, 1], BF16, tag="rzcT")
                nc.vector.tensor_copy(rzcT[:], rzcp[:])
                for c in range(CH):
                    ycls = ps_yz.tile([P, SC], F32, tag="yz")
                    for g in range(G):
                        for hf in range(2):
                            h = 2 * c + hf
                            e, hh = blk_of(h)
                            nc.tensor.matmul(
                                ycls[hf * HD : (hf + 1) * HD, 0:1],
                                lhsT=vtok[:SC, g, h * HD : (h + 1) * HD],
                                rhs=ucls[:, g, e * BH + hh : e * BH + hh + 1],
                                start=(g == 0),
                                stop=(g == G - 1),
                            )
                    rzbc = ps_tp.tile([P, 1], F32, tag="tp")
                    nc.tensor.matmul(
                        rzbc[:],
                        lhsT=selc[:, c * P : (c + 1) * P],
                        rhs=rzcT[:],
                        start=True,
                        stop=True,
                    )
                    rzbc_sb = zpool.tile([P, 1], BF16, tag="rzbc")
                    nc.vector.tensor_copy(rzbc_sb[:], rzbc[:])
                    nc.vector.tensor_tensor(
                        yT[:, c, 0:1],
                        ycls[:, 0:1],
                        rzbc_sb[:],
                        mybir.AluOpType.mult,
                    )

            prev = None
            pending_tail = None
            for b in range(NB):
                cb = b * S
                if b == 0:
                    xb = xb0
                else:
                    xb = xpool.tile([P, CH, S], BF16, tag="xb")
                    nc.sync.dma_start(
                        xb[:],
                        x_t[:, cb : cb + S].rearrange("(c p) s -> p c s", p=P),
                    )

                qT = qkpool.tile([P, CH, S], BF16, tag="qT")
                kT = qkpool.tile([P, CH, S], BF16, tag="kT")
                vtok = vpool.tile([P, G, C], BF16, tag="vtok")
                emit_qkv(xb, qT, kT, vtok)
                if b == 0:
                    # w_proj is first needed by out_proj(0), a batch later
                    for c in range(CH):
                        nc.sync.dma_start(wp_sb[:, c, :], wp_r[:, c, :])

                if pending_tail is not None:
                    pending_tail()
                    pending_tail = None
                if prev is not None:
                    emit_out_proj(*prev)
                    prev = None

                # ---- attention ----
                if b == NB - 1:
                    ob_last = opool.tile([P, CH, S], BF16, tag="ob", name="ob")
                else:
                    ob_last = None
                yT = ypool.tile([P, CH, S], BF16, tag="yT")
                zt = zpool.tile([SC, G, NH], F32, tag="zt")
                ucls = zpool.tile([SC, G, NH], BF16, tag="ucls")
                sps = {}   # block t -> sp3 psum tile
                us = {}    # block t -> u3 sbuf tile (exp'd scores)
                uts = {}   # block t -> ut3 sbuf tile

                def stageA(t, qT=qT, kT=kT, sps=sps):
                    g, e = t // NBLK, t % NBLK
                    k0 = g * SC
                    pq = 0 if e < 2 else HD    # PE quadrant of this block
                    sp3 = ps_sc.tile([SC, BH, SC], F32, tag="sc")
                    qe = qen if g == 0 else qed
                    # one accumulation group per PSUM bank: mask opens it over
                    # the whole tile, per-head scores accumulate, last closes.
                    # All matmuls of the block use the same PE quadrant.
                    nc.tensor.matmul(
                        sp3[:, :, :],
                        lhsT=qe[pq : pq + HD, :],
                        rhs=ke5[pq : pq + HD, :],
                        start=True,
                        stop=False,
                    )
                    for hh in range(BH):
                        h = head_of(e, hh)
                        c = h // 2
                        nc.tensor.matmul(
                            sp3[:, hh, :],
                            lhsT=qT[pq : pq + HD, c, k0 : k0 + SC],
                            rhs=kT[pq : pq + HD, c, k0 : k0 + SC],
                            start=False,
                            stop=(hh == BH - 1),
                        )
                    sps[t] = sp3

                def stageB1(t, zt=zt, sps=sps, us=us):
                    g, e = t // NBLK, t % NBLK
                    sp3 = sps.pop(t)
                    u3 = upool.tile([SC, BH, SC], BF16, tag="u3")
                    nc.scalar.activation(
                        u3[:, :, :],
                        sp3[:, :, :],
                        mybir.ActivationFunctionType.Exp,
                        bias=biasq[:],
                    )
                    nc.vector.tensor_reduce(
                        zt[:, g, e * BH : (e + 1) * BH],
                        u3[:, :, :],
                        mybir.AxisListType.X,
                        mybir.AluOpType.add,
                    )
                    us[t] = u3

                def stageB2(g, rzb, ucls=ucls, us=us, uts=uts,
                            blocks=range(NBLK)):
                    # normalize token-query rows in place (rzb row 0 is 1.0 so
                    # the cls row stays raw for the cross-slot cls softmax),
                    # then transpose per head
                    for e in blocks:
                        t = g * NBLK + e
                        u3 = us.pop(t)
                        neng = norm_eng if e % 2 else nc.vector
                        neng.tensor_tensor(
                            u3[:, :, :],
                            u3[:, :, :],
                            rzb[:, e * BH : (e + 1) * BH].to_broadcast(
                                [SC, BH, SC]
                            ),
                            mybir.AluOpType.mult,
                        )
                        tp3 = ps_tp.tile([SC, BH, P], BF16, tag="tp")
                        for hh in range(BH):
                            nc.tensor.transpose(
                                tp3[:, hh, :SC], u3[:, hh, :], identb[:SC, :SC]
                            )
                        ut3 = upool.tile([SC, BH, SC], BF16, tag="ut3", bufs=5)
                        if e % 2:
                            nc.scalar.copy(ut3[:, :, :], tp3[:, :, :SC])
                        else:
                            nc.vector.tensor_copy(ut3[:, :, :], tp3[:, :, :SC])
                        small_eng.tensor_copy(
                            ucls[:, g, e * BH : (e + 1) * BH], ut3[:, :, 0:1]
                        )
                        uts[t] = ut3

                def stageC(g, yT=yT, vtok=vtok, uts=uts, c0=0, pop=True):
                    # 3 feature chunks share one PSUM bank; chunks c0..c0+2
                    # consume only the two same-parity blocks transposed just
                    # before, and land in yT with a single DVE copy
                    k0 = g * SC
                    yz = ps_yz.tile([P, 3, SC], F32, tag="yz")
                    for ci in range(3):
                        c = c0 + ci
                        for hf in range(2):
                            h = 2 * c + hf
                            e, hh = blk_of(h)
                            nc.tensor.matmul(
                                yz[hf * HD : (hf + 1) * HD, ci, 0:SC],
                                lhsT=vtok[:SC, g, h * HD : (h + 1) * HD],
                                rhs=uts[g * NBLK + e][:, hh, :],
                                start=True,
                                stop=True,
                            )
                    nc.vector.tensor_copy(
                        yT[:, c0 : c0 + 3, k0 + 1 : k0 + SC], yz[:, :, 1:SC]
                    )
                    if pop:
                        for e in range(NBLK):
                            uts.pop(g * NBLK + e)

                NT = G * NBLK
                rzbs = {}
                for t in range(NT + 1):
                    if t < NT:
                        stageA(t)
                    if t >= 1:
                        stageB1(t - 1)
                        if (t - 1) % NBLK == NBLK - 1:
                            g = (t - 1) // NBLK
                            rzf = zpool.tile([SC, NH], F32, tag="rzf")
                            nc.vector.reciprocal(rzf[:], zt[:, g, :])
                            rzb = zpool.tile([SC, NH], BF16, tag="rzb")
                            small_eng.tensor_copy(rzb[:], rzf[:])
                            small_eng.memset(rzb[0:1, :], 1.0)
                            rzbs[g] = rzb
                            # B2/C of the previous slot: their inputs have had
                            # a full slot of slack, so PE never stalls on them
                            if g >= 1:
                                rzbp = rzbs.pop(g - 1)
                                stageB2(g - 1, rzbp, blocks=[0, 2])
                                stageC(g - 1, c0=0, pop=False)
                                stageB2(g - 1, rzbp, blocks=[1, 3])
                                stageC(g - 1, c0=3)
                                if b == NB - 1:
                                    # drain: project finished slots immediately
                                    k0 = (g - 1) * SC
                                    lo = k0 + (1 if g - 1 == 0 else 0)
                                    emit_out_proj_cols(
                                        yT, ob_last, cb, lo, k0 + SC - lo
                                    )

                def finish_tail(rzbs=rzbs, stageB2=stageB2, stageC=stageC,
                                zt=zt, ucls=ucls, yT=yT, vtok=vtok,
                                ob_last=ob_last, cb=cb, b=b):
                    rzbp = rzbs.pop(G - 1)
                    stageB2(G - 1, rzbp, blocks=[0, 2])
                    stageC(G - 1, c0=0, pop=False)
                    stageB2(G - 1, rzbp, blocks=[1, 3])
                    stageC(G - 1, c0=3)
                    if b == NB - 1:
                        k0 = (G - 1) * SC
                        emit_out_proj_cols(yT, ob_last, cb, k0, SC)
                    emit_cls(zt, ucls, yT, vtok)
                    if b == NB - 1:
                        emit_out_proj_cols(yT, ob_last, cb, 0, 1)

                pending_tail = finish_tail

                if b < NB - 1:
                    ob = opool.tile([P, CH, S], BF16, tag="ob")
                    prev = (yT, ob, cb)

            pending_tail()

    nc.compile()
    return nc


_NC_CACHE = None
_LAST_IN_MAPS = None


def kernel(x, w_qkv, w_proj, b_proj):
    global _NC_CACHE, _LAST_IN_MAPS
    x = np.asarray(x)
    w_qkv = np.asarray(w_qkv)
    w_proj = np.asarray(w_proj)
    b_proj = np.asarray(b_proj)

    perm, valid = _perm_valid()
    qen, qed, ke5, biasq, selc = _consts()

    wq = np.array(w_qkv, np.float32, copy=True)
    wq[:, :C] *= 1.0 / np.sqrt(HD)
    wq = wq.astype(BFNP)
    wp = w_proj.astype(BFNP)
    b_pc = np.ascontiguousarray(b_proj.astype(np.float32).reshape(CH, P).T)

    in_maps = []
    for core in range(NCORES):
        xs = x[core * NB : (core + 1) * NB]          # (NB, 785, C)
        xp = xs[:, perm, :]                          # (NB, S, C)
        x_T = np.ascontiguousarray(
            xp.transpose(2, 0, 1).reshape(C, TT)
        ).astype(BFNP)
        in_maps.append(
            {
                "x_t": x_T,
                "w_qkv": wq,
                "w_proj": wp,
                "b_pc": b_pc,
                "qen": qen,
                "qed": qed,
                "ke5": ke5,
                "biasq": biasq,
                "selc": selc,
            }
        )

    if _NC_CACHE is None:
        _NC_CACHE = build_bass()
    nc = _NC_CACHE

    _LAST_IN_MAPS = in_maps

    res = run_bass_kernel_spmd(nc, in_maps, core_ids=list(range(NCORES)))

    out = np.zeros((B_TOTAL, N_TOK, C), np.float32)
    vperm = perm[valid]
    for core in range(NCORES):
        o_t = np.asarray(res.results[core]["out_t"]).astype(np.float32)
        op = o_t.reshape(C, NB, S).transpose(1, 2, 0)  # (NB, S, C)
        out[core * NB : (core + 1) * NB][:, vperm, :] = op[:, valid, :]
    return out


if __name__ == "__main__":
    rng = np.random.default_rng(0)
    x = rng.standard_normal((B_TOTAL, N_TOK, C)).astype(np.float32)
    w_qkv = (rng.standard_normal((C, 3 * C)) * 0.02).astype(np.float32)
    w_proj = (rng.standard_normal((C, C)) * 0.02).astype(np.float32)
    b_proj = np.zeros((C,), np.float32)
    y = kernel(x=x, w_qkv=w_qkv, w_proj=w_proj, b_proj=b_proj)
    print(np.abs(y).mean(), y.shape, y.dtype)
